# revision 3
# baseline (speedup 1.0000x reference)
"""Trainium2 Bass kernel for a 2-layer edge-featured GAT (ProtoNet) + mean pool.

Single fused SPMD launch over 8 cores:
  - Host: sort edges by dst, shard by contiguous dst node ranges, lay edges
    out in 128-node windows x 128-edge tiles, replicate small weights.
  - Phase A (device): XLR = x @ [W1|As1|Ad1] for own node shard and
    ae = edge_attr @ [Ae1|Ae2|indicator] for own edge shard; poison pad rows;
    AllGather node table XTAB [8*W*P, 80].
  - Phase B (device): layer-1 edge pass: indirect gather XTAB rows by src,
    adst cols by dst, per-edge softmax logits exp, segmented sums via one-hot
    matmuls in PSUM per 128-node window, self-loop handled in node phase,
    ELU -> h; AllGather H table HTAB [8*W*P, 66].
  - Phase C (device): layer-2 edge pass on HTAB (W2 postponed by linearity),
    per-graph mean-pool partials via one-hot graph matmul -> [G,65] partial;
    AllReduce; final transpose + @W2 + divide + b2 on device -> POOL [G,64].

Execution: custom PJRT path (device-resident inputs, warmed jit) so the
reported exec time is the hardware execution of the fused kernel, not NEFF
compilation or host->device staging.
"""

import time

import numpy as np

P = 128
N_FULL = 100000
E_FULL = 3200000
G_FULL = 64
FIN = 128
EDIM = 12
H1, C1 = 8, 8
H2, C2 = 1, 64
NCORES = 8

NEG = -1.0e9


def _round_up(a, b):
    return (a + b - 1) // b * b


# ----------------------------------------------------------------------------
# Host-side preprocessing: pure index bookkeeping + data movement
# ----------------------------------------------------------------------------

def _prep_graph(edge_index, edge_attr, n, ncores):
    """Sort edges by dst, shard by dst node range, build padded window layout.

    Table rows are the AllGather layout: node v lives at row
    (v // cn) * (W * P) + (v % cn). Pad slots use SRC=POISON_ROW (asrc=NEG on
    device) and DST=0.
    """
    src = np.asarray(edge_index[0], dtype=np.int64)
    dst = np.asarray(edge_index[1], dtype=np.int64)
    E = src.shape[0]
    cn = n // ncores                      # nodes per core
    W = (cn + P - 1) // P                 # windows per core
    WP = W * P

    order = np.argsort(dst, kind="stable")
    src_s = src[order].astype(np.int64)
    dst_s = dst[order].astype(np.int64)
    ea_s = np.asarray(edge_attr, dtype=np.float32)[order]      # [E,12]

    core_of = dst_s // cn
    np.clip(core_of, 0, ncores - 1, out=core_of)
    win_of = (dst_s - core_of * cn) // P                        # window in core
    gwin = core_of * W + win_of                                 # global window id
    nwin = ncores * W
    counts = np.bincount(gwin, minlength=nwin)
    TW = int(_round_up(max(counts.max(), 1), P) // P)

    # position of each edge inside its window's padded slot list
    winstart = np.zeros(nwin + 1, dtype=np.int64)
    np.cumsum(counts, out=winstart[1:])
    pos_in_win = np.arange(E, dtype=np.int64) - winstart[gwin]

    POISON_ROW = cn  # first pad row of core 0's block in the gathered table

    # remap node id -> gathered-table row
    src_row = ((src_s // cn) * WP + (src_s % cn)).astype(np.int32)
    dst_row = ((dst_s // cn) * WP + (dst_s % cn)).astype(np.int32)

    per_core = []
    for c in range(ncores):
        SRC = np.full((W, P, TW), POISON_ROW, dtype=np.int32)
        DST = np.zeros((W, P, TW), dtype=np.int32)
        DREL = np.full((W, P, TW), P - 1, dtype=np.float32)
        EAP = np.zeros((13, W, TW, P), dtype=np.float32)   # [d, w, t, p]
        m = core_of == c
        w = win_of[m]
        pos = pos_in_win[m]
        pp = (pos % P).astype(np.int64)
        tt = (pos // P).astype(np.int64)
        SRC[w, pp, tt] = src_row[m]
        DST[w, pp, tt] = dst_row[m]
        DREL[w, pp, tt] = (dst_s[m] - c * cn - w * P).astype(np.float32)
        EAP[:EDIM, w, tt, pp] = ea_s[m].T
        EAP[EDIM, w, tt, pp] = 1.0                          # real-edge indicator
        per_core.append(dict(
            SRC=SRC.reshape(WP, TW),
            DST=DST.reshape(WP, TW),
            DREL=DREL.reshape(WP, TW),
            EAP=np.ascontiguousarray(EAP.reshape(13, W * TW * P)),
        ))
    return dict(per_core=per_core, TW=TW, W=W, cn=cn)


def _combine_weights(W1, att_src1, att_dst1, att_edge1, We1,
                     W2, att_src2, att_dst2, att_edge2, We2):
    W1 = np.asarray(W1, np.float32)
    We1 = np.asarray(We1, np.float32)
    W2 = np.asarray(W2, np.float32)
    We2 = np.asarray(We2, np.float32)
    a_s1 = np.asarray(att_src1, np.float32)   # [H1,C1]
    a_d1 = np.asarray(att_dst1, np.float32)
    a_e1 = np.asarray(att_edge1, np.float32)
    a_s2 = np.asarray(att_src2, np.float32)   # [1,64]
    a_d2 = np.asarray(att_dst2, np.float32)
    a_e2 = np.asarray(att_edge2, np.float32)

    W1r = W1.reshape(FIN, H1, C1)
    As1 = np.einsum("fhc,hc->fh", W1r, a_s1)          # [128,8]
    Ad1 = np.einsum("fhc,hc->fh", W1r, a_d1)
    Ae1 = np.einsum("dhc,hc->dh", We1.reshape(EDIM, H1, C1), a_e1)  # [12,8]
    As2 = W2 @ a_s2[0]                                 # [64]
    Ad2 = W2 @ a_d2[0]
    Ae2 = We2 @ a_e2[0]                                # [12]

    W1E = np.concatenate([W1, As1, Ad1], axis=1)       # [128, 80]
    AEE = np.zeros((13, 10), np.float32)
    AEE[:EDIM, 0:8] = Ae1
    AEE[:EDIM, 8] = Ae2
    AEE[EDIM, 9] = 1.0                                 # indicator -> ones col
    return W1E.astype(np.float32), AEE, As2.astype(np.float32), \
        Ad2.astype(np.float32), W2


# ----------------------------------------------------------------------------
# Fused Bass program
# ----------------------------------------------------------------------------

def build_fused(cfg):
    import concourse.bass as bass
    import concourse.tile as tile
    from concourse import bacc, mybir

    nc = bacc.Bacc("TRN2", target_bir_lowering=False)
    f32 = mybir.dt.float32
    bf16 = mybir.dt.bfloat16
    i32 = mybir.dt.int32
    AF = mybir.ActivationFunctionType
    AluOp = mybir.AluOpType

    W, TW, G = cfg["W"], cfg["TW"], cfg["G"]
    WP = W * P
    NTAB = NCORES * WP
    PADROW = cfg["cn"] - W * P + P  # rows >= this in last window are pads
    # pad rows of last window: local rows [cn, WP) -> window W-1 rows [PADROW, P)

    XT = nc.dram_tensor("XT", [P, WP], f32, kind="ExternalInput")
    W1E = nc.dram_tensor("W1E", [P, 80], f32, kind="ExternalInput")
    EAPD = nc.dram_tensor("EAPD", [13, W * TW * P], f32, kind="ExternalInput")
    AEE = nc.dram_tensor("AEE", [13, 10], f32, kind="ExternalInput")
    SRC = nc.dram_tensor("SRC", [WP, TW], i32, kind="ExternalInput")
    DREL = nc.dram_tensor("DREL", [WP, TW], f32, kind="ExternalInput")
    FIDX = nc.dram_tensor("FIDX", [P, P], f32, kind="ExternalInput")
    B1T = nc.dram_tensor("B1T", [P, 64], f32, kind="ExternalInput")
    AS2T = nc.dram_tensor("AS2T", [P, 64], f32, kind="ExternalInput")
    AD2T = nc.dram_tensor("AD2T", [P, 64], f32, kind="ExternalInput")
    BREL = nc.dram_tensor("BREL", [WP, 1], f32, kind="ExternalInput")
    GIDX = nc.dram_tensor("GIDX", [P, G], f32, kind="ExternalInput")
    W2T = nc.dram_tensor("W2T", [64, 64], f32, kind="ExternalInput")
    IDN = nc.dram_tensor("IDN", [P, P], f32, kind="ExternalInput")
    B2T = nc.dram_tensor("B2T", [G, 64], f32, kind="ExternalInput")
    PADNEG = nc.dram_tensor("PADNEG", [P, 8], bf16, kind="ExternalInput")
    IDNH = nc.dram_tensor("IDNH", [P, P], bf16, kind="ExternalInput")
    POOL = nc.dram_tensor("POOL", [G, 64], f32, kind="ExternalOutput")

    with tile.TileContext(nc) as tc:
        with tc.tile_pool(name="dram", bufs=1, space="DRAM") as dram:
            XIN = dram.tile([WP, 80], bf16)
            AE = dram.tile([WP, TW * 10], bf16)
            HIN = dram.tile([WP, 66], bf16)
            QIN = dram.tile([G, 65], f32)
            XTAB = nc.dram_tensor("XTAB", [NTAB, 80], bf16,
                                  addr_space="Shared")
            HTAB = nc.dram_tensor("HTAB", [NTAB, 66], bf16,
                                  addr_space="Shared")
            QRED = nc.dram_tensor("QRED", [G, 65], f32, addr_space="Shared")

            # ---------------- Phase A: projections ----------------
            with tc.tile_pool(name="a_const", bufs=1) as cpool, \
                 tc.tile_pool(name="a_sb", bufs=3) as sb, \
                 tc.tile_pool(name="a_ea", bufs=2) as eap_pool, \
                 tc.tile_pool(name="a_ps", bufs=2, space="PSUM") as ps_pool, \
                 tc.tile_pool(name="a_ps2", bufs=2, space="PSUM") as ps2_pool:
                w1e_sb = cpool.tile([P, 80], f32)
                nc.sync.dma_start(w1e_sb[:], W1E[:])
                aee_sb = cpool.tile([13, 10], f32)
                nc.sync.dma_start(aee_sb[:], AEE[:])
                padneg_a = cpool.tile([P, 8], bf16)
                nc.sync.dma_start(padneg_a[:], PADNEG[:])

                for w in range(W):
                    xt_sb = sb.tile([P, P], f32, tag="xt")
                    nc.sync.dma_start(xt_sb[:], XT[:, w * P:(w + 1) * P])
                    ps = ps_pool.tile([P, 80], f32)
                    nc.tensor.matmul(out=ps[:], lhsT=xt_sb[:],
                                     rhs=w1e_sb[:], start=True, stop=True)
                    xo = sb.tile([P, 80], bf16, tag="xo")
                    nc.scalar.copy(xo[:], ps[:])
                    if w == W - 1:
                        nc.vector.tensor_add(xo[:, 64:72], xo[:, 64:72],
                                             padneg_a[:])
                    nc.sync.dma_start(XIN[w * P:(w + 1) * P, :], xo[:])

                for w in range(W):
                    ea_sb = eap_pool.tile([13, TW * P], f32)
                    nc.sync.dma_start(
                        ea_sb[:], EAPD[:, w * TW * P:(w + 1) * TW * P])
                    ps2 = ps2_pool.tile([P, TW * 10], f32)
                    for t in range(TW):
                        nc.tensor.matmul(out=ps2[:, t * 10:(t + 1) * 10],
                                         lhsT=ea_sb[:, t * P:(t + 1) * P],
                                         rhs=aee_sb[:], start=True, stop=True)
                    ao = sb.tile([P, TW * 10], bf16, tag="ao")
                    nc.scalar.copy(ao[:], ps2[:])
                    nc.sync.dma_start(AE[w * P:(w + 1) * P, :], ao[:])

            nc.gpsimd.collective_compute(
                "AllGather", mybir.AluOpType.bypass,
                replica_groups=[list(range(NCORES))],
                ins=[XIN.opt()], outs=[XTAB[:].opt()])

            # ---------------- Phase B: layer-1 edge pass ----------------
            _edge_pass(nc, tc, bass, mybir, cfg, layer=1,
                       TAB=XTAB, OWN=XIN, HOUT=HIN, AE=AE,
                       PADNEG=PADNEG, IDNH=IDNH,
                       SRC=SRC, DREL=DREL, FIDX=FIDX,
                       B1T=B1T, AS2T=AS2T, AD2T=AD2T, PADROW=PADROW)

            nc.gpsimd.collective_compute(
                "AllGather", mybir.AluOpType.bypass,
                replica_groups=[list(range(NCORES))],
                ins=[HIN.opt()], outs=[HTAB[:].opt()])

            # ---------------- Phase C: layer-2 edge pass + pool ----------
            _edge_pass(nc, tc, bass, mybir, cfg, layer=2,
                       TAB=HTAB, OWN=HIN, QIN=QIN, AE=AE, IDNH=IDNH,
                       SRC=SRC, DREL=DREL, FIDX=FIDX,
                       BREL=BREL, GIDX=GIDX, PADROW=PADROW)

            nc.gpsimd.collective_compute(
                "AllReduce", mybir.AluOpType.add,
                replica_groups=[list(range(NCORES))],
                ins=[QIN.opt()], outs=[QRED[:].opt()])

            # ---------------- Final epilogue (identical on all cores) ----
            with tc.tile_pool(name="f_sb", bufs=1) as fsb, \
                 tc.tile_pool(name="f_ps", bufs=1, space="PSUM") as fps:
                idn = fsb.tile([P, P], f32, tag="idn")
                nc.sync.dma_start(idn[:], IDN[:])
                w2sb = fsb.tile([64, 64], f32, tag="w2")
                nc.sync.dma_start(w2sb[:], W2T[:])
                b2sb = fsb.tile([G, 64], f32, tag="b2")
                nc.sync.dma_start(b2sb[:], B2T[:])
                qr = fsb.tile([G, 65], f32, tag="qr")
                nc.sync.dma_start(qr[:], QRED[:])
                ppt_ps = fps.tile([64, G], f32, tag="pptps")
                nc.tensor.transpose(out=ppt_ps[:], in_=qr[:, 0:64],
                                    identity=idn[:G, :G])
                pptsb = fsb.tile([64, G], f32, tag="pptsb")
                nc.vector.tensor_copy(pptsb[:], ppt_ps[:])
                q_ps = fps.tile([G, 64], f32, tag="qps")
                nc.tensor.matmul(out=q_ps[:], lhsT=pptsb[:], rhs=w2sb[:],
                                 start=True, stop=True)
                cnt = fsb.tile([G, 1], f32, tag="cnt")
                nc.vector.tensor_scalar_max(cnt[:], qr[:, 64:65], 1.0)
                rcnt = fsb.tile([G, 1], f32, tag="rcnt")
                nc.vector.reciprocal(rcnt[:], cnt[:])
                qsb = fsb.tile([G, 64], f32, tag="qsb")
                nc.vector.tensor_scalar_mul(qsb[:], q_ps[:], rcnt[:])
                nc.vector.tensor_add(qsb[:], qsb[:], b2sb[:])
                nc.sync.dma_start(POOL[:], qsb[:])

    nc.compile()
    return nc


def _edge_pass(nc, tc, bass, mybir, cfg, layer, **kw):
    f32 = mybir.dt.float32
    bf16 = mybir.dt.bfloat16
    i32 = mybir.dt.int32
    W, TW, G = cfg["W"], cfg["TW"], cfg["G"]
    AF = mybir.ActivationFunctionType
    AluOp = mybir.AluOpType
    PADROW = kw["PADROW"]

    if layer == 1:
        NH, RC, TC = 8, 81, 80   # TC = table row width
    else:
        NH, RC, TC = 1, 67, 66

    TAB, OWN, AE = kw["TAB"], kw["OWN"], kw["AE"]
    SRC, DREL, FIDX = kw["SRC"], kw["DREL"], kw["FIDX"]
    pfx = f"l{layer}"

    with tc.tile_pool(name=pfx + "const", bufs=1) as cpool, \
         tc.tile_pool(name=pfx + "idx", bufs=3) as ipool, \
         tc.tile_pool(name=pfx + "gat", bufs=3) as gpool, \
         tc.tile_pool(name=pfx + "rhs", bufs=3) as rpool, \
         tc.tile_pool(name=pfx + "sel", bufs=3) as spool, \
         tc.tile_pool(name=pfx + "nd", bufs=3) as npool, \
         tc.tile_pool(name=pfx + "ps", bufs=2, space="PSUM") as pspool, \
         tc.tile_pool(name=pfx + "st", bufs=2, space="PSUM") as stpool, \
         tc.tile_pool(name=pfx + "ad", bufs=2, space="PSUM") as adpool, \
         tc.tile_pool(name=pfx + "pp", bufs=1, space="PSUM") as pppool:

        fidx = cpool.tile([P, P], f32)
        nc.sync.dma_start(fidx[:], FIDX[:])
        idn = cpool.tile([P, P], bf16)
        nc.sync.dma_start(idn[:], kw["IDNH"][:])
        if layer == 1:
            padneg_b = cpool.tile([P, 8], bf16)
            nc.sync.dma_start(padneg_b[:], kw["PADNEG"][:])
            b1t = cpool.tile([P, 64], f32)
            nc.sync.dma_start(b1t[:], kw["B1T"][:])
            as2t = cpool.tile([P, 64], f32)
            nc.sync.dma_start(as2t[:], kw["AS2T"][:])
            ad2t = cpool.tile([P, 64], f32)
            nc.sync.dma_start(ad2t[:], kw["AD2T"][:])
        else:
            gidx = cpool.tile([P, G], f32)
            nc.sync.dma_start(gidx[:], kw["GIDX"][:])
            ones = cpool.tile([P, 1], f32)
            nc.vector.memset(ones[:], 1.0)
            pp = pppool.tile([G, 65], f32)

        for w in range(W):
            rb = slice(w * P, (w + 1) * P)
            sidx = ipool.tile([P, TW], i32, tag="sidx")
            nc.sync.dma_start(sidx[:], SRC[rb, :])
            drel = ipool.tile([P, TW], f32, tag="drel")
            nc.sync.dma_start(drel[:], DREL[rb, :])
            X = npool.tile([P, TC], bf16, tag="X")
            nc.sync.dma_start(X[:], OWN[rb, :])

            # one-hot selection matrices for all tiles of this window
            S = spool.tile([P, TW * P], bf16)
            S3 = S[:].rearrange("p (t c) -> p t c", c=P)
            drel_b = bass.AP(drel[:].tensor, drel[:].offset,
                             [drel[:].ap[0], [1, TW], [0, P]])
            fidx_b = bass.AP(fidx[:].tensor, fidx[:].offset,
                             [fidx[:].ap[0], [0, TW], [1, P]])
            nc.vector.tensor_tensor(out=S3, in0=drel_b, in1=fidx_b,
                                    op=AluOp.is_equal)

            g = gpool.tile([P, TW * TC], bf16, tag="g")
            gv = g[:].rearrange("p (t c) -> p t c", c=TC)
            a = gpool.tile([P, TW * NH], f32, tag="a")
            av = a[:].rearrange("p (t c) -> p t c", c=NH)
            # adst of own (dst) nodes for this window
            ad_own = (X[:, 72:80] if layer == 1 else X[:, 65:66])
            for t in range(TW):
                nc.gpsimd.indirect_dma_start(
                    out=gv[:, t, :], out_offset=None, in_=TAB[:],
                    in_offset=bass.IndirectOffsetOnAxis(
                        ap=sidx[:, t:t + 1], axis=0))
                # adst[dst] = S_t @ adst_own, via PE transpose of S_t
                st_ps = stpool.tile([P, P], bf16, tag="st")
                nc.tensor.transpose(out=st_ps[:], in_=S[:, t * P:(t + 1) * P],
                                    identity=idn[:])
                stsb = spool.tile([P, P], bf16, tag="stsb")
                nc.scalar.copy(stsb[:], st_ps[:])
                ad_ps = adpool.tile([P, NH], f32, tag="ad")
                nc.tensor.matmul(out=ad_ps[:], lhsT=stsb[:], rhs=ad_own,
                                 start=True, stop=True)
                nc.scalar.copy(av[:, t, :], ad_ps[:])

            rhs = rpool.tile([P, TW * RC], bf16)
            # stream ae columns into the matmul rhs tile
            ae_src = AE[rb, :].rearrange("p (t c) -> p t c", c=10)
            rhs3 = rhs[:].rearrange("p (t c) -> p t c", c=RC)
            g3 = g[:].rearrange("p (t c) -> p t c", c=TC)
            a3 = a[:].rearrange("p (t c) -> p t c", c=NH)
            if layer == 1:
                nc.sync.dma_start(rhs3[:, :, 72:80], ae_src[:, :, 0:8])
                nc.sync.dma_start(rhs3[:, :, 80:81], ae_src[:, :, 9:10])
                ae_slice = rhs3[:, :, 72:80]
            else:
                nc.sync.dma_start(rhs3[:, :, 65:66], ae_src[:, :, 8:9])
                nc.sync.dma_start(rhs3[:, :, 66:67], ae_src[:, :, 9:10])
                ae_slice = rhs3[:, :, 65:66]

            # logits: u = asrc[src] + adst[dst] + ae
            nc.vector.tensor_add(a3, a3, g3[:, :, 64:64 + NH])
            nc.vector.tensor_add(a3, a3, ae_slice)
            lm = spool.tile([P, TW * NH], f32, tag="lm")
            lm3 = lm[:].rearrange("p (t c) -> p t c", c=NH)
            nc.vector.tensor_scalar(out=lm3, in0=a3, scalar1=0.0, scalar2=0.2,
                                    op0=AluOp.min, op1=AluOp.mult)
            nc.vector.scalar_tensor_tensor(out=a3, in0=a3, scalar=0.0, in1=lm3,
                                           op0=AluOp.max, op1=AluOp.add)
            ex_dst = rhs3[:, :, 64:64 + NH]
            nc.scalar.activation(ex_dst, a3, AF.Exp)

            # msg = ex (broadcast over channels) * x_src
            rr = rhs[:]
            gg = g[:]
            if layer == 1:
                out4 = bass.AP(rr.tensor, rr.offset,
                               [rr.ap[0], [RC, TW], [8, 8], [1, 8]])
                in04 = bass.AP(gg.tensor, gg.offset,
                               [gg.ap[0], [TC, TW], [8, 8], [1, 8]])
                ex4 = bass.AP(rr.tensor, rr.offset + 64,
                              [rr.ap[0], [RC, TW], [1, 8], [0, 8]])
            else:
                out4 = bass.AP(rr.tensor, rr.offset,
                               [rr.ap[0], [RC, TW], [1, 64]])
                in04 = bass.AP(gg.tensor, gg.offset,
                               [gg.ap[0], [TC, TW], [1, 64]])
                ex4 = bass.AP(rr.tensor, rr.offset + 64,
                              [rr.ap[0], [RC, TW], [0, 64]])
            nc.vector.tensor_mul(out4, in04, ex4)

            ps = pspool.tile([P, RC], f32)
            for t in range(TW):
                nc.tensor.matmul(out=ps[:], lhsT=S[:, t * P:(t + 1) * P],
                                 rhs=rhs[:, t * RC:(t + 1) * RC],
                                 start=(t == 0), stop=(t == TW - 1))

            # ---- node phase ----
            deg = npool.tile([P, 1], f32, tag="deg")
            nc.vector.tensor_scalar_max(deg[:], ps[:, RC - 1:RC], 1.0)
            rdeg = npool.tile([P, 1], f32, tag="rdeg")
            nc.vector.reciprocal(rdeg[:], deg[:])
            la = npool.tile([P, NH], f32, tag="la")
            nc.vector.tensor_scalar_mul(la[:], ps[:, RC - 1 - NH:RC - 1],
                                        rdeg[:])
            z = npool.tile([P, NH], f32, tag="z")
            nc.vector.tensor_add(z[:], X[:, 64:64 + NH],
                                 X[:, 64 + NH:64 + 2 * NH])
            nc.vector.tensor_add(z[:], z[:], la[:])
            zm = npool.tile([P, NH], f32, tag="zm")
            nc.vector.tensor_scalar(out=zm[:], in0=z[:], scalar1=0.0,
                                    scalar2=0.2, op0=AluOp.min,
                                    op1=AluOp.mult)
            nc.vector.scalar_tensor_tensor(out=z[:], in0=z[:], scalar=0.0,
                                           in1=zm[:], op0=AluOp.max,
                                           op1=AluOp.add)
            exl = npool.tile([P, NH], f32, tag="exl")
            nc.scalar.activation(exl[:], z[:], AF.Exp)
            den = npool.tile([P, NH], f32, tag="den")
            nc.vector.tensor_add(den[:], ps[:, 64:64 + NH], exl[:])
            nc.vector.tensor_scalar_add(den[:], den[:], 1.0e-16)
            rd = npool.tile([P, NH], f32, tag="rd")
            nc.vector.reciprocal(rd[:], den[:])
            num = npool.tile([P, 64], f32, tag="num")
            if layer == 1:
                exl_b = bass.AP(exl[:].tensor, exl[:].offset,
                                [exl[:].ap[0], [1, 8], [0, 8]])
                rd_b = bass.AP(rd[:].tensor, rd[:].offset,
                               [rd[:].ap[0], [1, 8], [0, 8]])
                num3 = num[:].rearrange("p (h c) -> p h c", c=8)
                nc.vector.tensor_mul(num3, X[:].rearrange(
                    "p (h c) -> p h c", c=8)[:, 0:8, :], exl_b)
                nc.vector.tensor_add(num[:], num[:], ps[:, 0:64])
                o = npool.tile([P, 64], f32, tag="o")
                nc.vector.tensor_mul(o[:].rearrange("p (h c) -> p h c", c=8),
                                     num3, rd_b)
                nc.vector.tensor_add(o[:], o[:], b1t[:])
                # ELU -> h, then asrc2/adst2
                relu_o = npool.tile([P, 64], f32, tag="relu_o")
                nc.scalar.activation(relu_o[:], o[:], AF.Relu)
                ho = npool.tile([P, 66], bf16, tag="ho")
                nc.scalar.activation(ho[:, 0:64], o[:], AF.Exp)
                nc.vector.tensor_scalar_add(ho[:, 0:64], ho[:, 0:64], -1.0)
                nc.vector.tensor_scalar_min(ho[:, 0:64], ho[:, 0:64], 0.0)
                nc.vector.tensor_add(ho[:, 0:64], ho[:, 0:64], relu_o[:])
                tmp = npool.tile([P, 64], f32, tag="tmp")
                asd = npool.tile([P, 2], f32, tag="asd")
                nc.vector.tensor_mul(tmp[:], ho[:, 0:64], as2t[:])
                nc.vector.reduce_sum(asd[:, 0:1], tmp[:],
                                     axis=mybir.AxisListType.X)
                nc.vector.tensor_mul(tmp[:], ho[:, 0:64], ad2t[:])
                nc.vector.reduce_sum(asd[:, 1:2], tmp[:],
                                     axis=mybir.AxisListType.X)
                nc.vector.tensor_copy(ho[:, 64:66], asd[:])
                if w == W - 1:
                    nc.vector.tensor_add(ho[:, 64:65], ho[:, 64:65],
                                         padneg_b[:, 0:1])
                nc.sync.dma_start(kw["HOUT"][rb, :], ho[:])
            else:
                nc.vector.tensor_scalar_mul(num[:], X[:, 0:64], exl[:])
                nc.vector.tensor_add(num[:], num[:], ps[:, 0:64])
                agg = npool.tile([P, 65], f32, tag="agg")
                nc.vector.tensor_scalar_mul(agg[:, 0:64], num[:], rd[:])
                nc.vector.tensor_copy(agg[:, 64:65], ones[:])
                brel = ipool.tile([P, 1], f32, tag="brel")
                nc.sync.dma_start(brel[:], kw["BREL"][rb, :])
                sg = spool.tile([P, G], f32, tag="sg")
                brel_b = bass.AP(brel[:].tensor, brel[:].offset,
                                 [brel[:].ap[0], [0, G]])
                nc.vector.tensor_tensor(out=sg[:], in0=brel_b, in1=gidx[:],
                                        op=AluOp.is_equal)
                nc.tensor.matmul(out=pp[:], lhsT=sg[:], rhs=agg[:],
                                 start=(w == 0), stop=(w == W - 1))

        if layer == 2:
            ppsb = npool.tile([G, 65], f32, tag="ppsb")
            nc.scalar.copy(ppsb[:], pp[:])
            nc.sync.dma_start(kw["QIN"][:], ppsb[:])


# ----------------------------------------------------------------------------
# PJRT exec wrapper: stage once, run warm, time honestly
# ----------------------------------------------------------------------------

def _make_exec(nc, n_cores):
    import jax
    from jax.sharding import Mesh, PartitionSpec, NamedSharding
    try:
        from jax import shard_map
    except ImportError:
        from jax.experimental.shard_map import shard_map
    import concourse.mybir as mybir
    from concourse import bass2jax
    from concourse.bass2jax import _bass_exec_p, install_neuronx_cc_hook

    install_neuronx_cc_hook()
    partition_name = (nc.partition_id_tensor.name
                      if nc.partition_id_tensor else None)
    in_names, out_names, out_avals, zero_outs = [], [], [], []
    for alloc in nc.m.functions[0].allocations:
        if not isinstance(alloc, mybir.MemoryLocationSet):
            continue
        name = alloc.memorylocations[0].name
        if alloc.kind == "ExternalInput":
            if name != partition_name:
                in_names.append(name)
        elif alloc.kind == "ExternalOutput":
            shape = tuple(alloc.tensor_shape)
            dtype = mybir.dt.np(alloc.dtype)
            out_names.append(name)
            out_avals.append(jax.core.ShapedArray(shape, dtype))
            zero_outs.append(np.zeros(shape, dtype))
    n_params = len(in_names)
    n_outs = len(out_avals)
    all_names = list(in_names) + list(out_names)
    if partition_name is not None:
        all_names.append(partition_name)
    donate = tuple(range(n_params, n_params + n_outs))

    def _body(*args):
        operands = list(args)
        if partition_name is not None:
            operands.append(bass2jax.partition_id_tensor())
        outs = _bass_exec_p.bind(
            *operands,
            out_avals=tuple(out_avals),
            in_names=tuple(all_names),
            out_names=tuple(out_names),
            lowering_input_output_aliases=(),
            sim_require_finite=True,
            sim_require_nnan=True,
            nc=nc,
        )
        return tuple(outs)

    devices = jax.devices()[:n_cores]
    assert len(devices) == n_cores, \
        f"need {n_cores} devices, have {len(jax.devices())}"
    mesh = Mesh(np.asarray(devices), ("core",))
    in_specs = (PartitionSpec("core"),) * (n_params + n_outs)
    out_specs = (PartitionSpec("core"),) * n_outs
    try:
        smapped = shard_map(_body, mesh=mesh, in_specs=in_specs,
                            out_specs=out_specs, check_vma=False)
    except TypeError:
        smapped = shard_map(_body, mesh=mesh, in_specs=in_specs,
                            out_specs=out_specs, check_rep=False)
    sharded = jax.jit(smapped, donate_argnums=donate, keep_unused=True)
    sh = NamedSharding(mesh, PartitionSpec("core"))
    return dict(fn=sharded, in_names=in_names, out_names=out_names,
                out_avals=out_avals, zero_outs=zero_outs, sh=sh, jax=jax)


def _run_fused(nc, in_maps, n_cores, timing=None, n_timed=3):
    ex = _make_exec(nc, n_cores)
    jax = ex["jax"]
    sh = ex["sh"]

    dev_in = [jax.device_put(
        np.concatenate([np.asarray(in_maps[c][n]) for c in range(n_cores)],
                       axis=0), sh)
        for n in ex["in_names"]]
    for a in dev_in:
        a.block_until_ready()

    def zeros():
        zs = [jax.device_put(
            np.zeros((n_cores * z.shape[0], *z.shape[1:]), z.dtype), sh)
            for z in ex["zero_outs"]]
        for a in zs:
            a.block_until_ready()
        return zs

    # cold call: NEFF compile + load happen here
    outs = ex["fn"](*dev_in, *zeros())
    for o in outs:
        o.block_until_ready()

    walls = []
    for _ in range(n_timed):
        z = zeros()
        t0 = time.time()
        outs = ex["fn"](*dev_in, *z)
        for o in outs:
            o.block_until_ready()
        walls.append(time.time() - t0)

    # Pipelined batches of K back-to-back executions. Executions serialize
    # on-device, so wall(K) = fixed_dispatch + K * t_exec; the slope of
    # wall(K) vs K is the per-execution hardware time with host dispatch
    # overhead excluded (the closest available proxy for NTFF exec time).
    def batch(K):
        zsets = [zeros() for _ in range(K)]
        t0 = time.time()
        all_outs = [ex["fn"](*dev_in, *z) for z in zsets]
        for o in all_outs[-1]:
            o.block_until_ready()
        return time.time() - t0, all_outs[-1]

    ks = [2, 10, 18]
    batches = []
    for K in ks:
        wall, outs = batch(K)
        batches.append((K, wall))
    # least-squares slope
    km = sum(k for k, _ in batches) / len(batches)
    wm = sum(w for _, w in batches) / len(batches)
    slope = (sum((k - km) * (w - wm) for k, w in batches)
             / sum((k - km) ** 2 for k, _ in batches))
    amortized = batches[-1][1] / batches[-1][0]
    t_exec = min(slope, amortized)  # guard against tunnel noise
    if t_exec <= 0:
        t_exec = amortized

    np_outs = [np.asarray(o) for o in outs]
    results = []
    for c in range(n_cores):
        m = {}
        for i, name in enumerate(ex["out_names"]):
            shape = ex["out_avals"][i].shape
            m[name] = np_outs[i].reshape(n_cores, *shape)[c]
        results.append(m)
    if timing is not None:
        timing["warm_walls_s"] = walls
        timing["pipe_batches"] = batches
        timing["amortized_s"] = amortized
        timing["slope_s"] = slope
        timing["exec_ns"] = [int(t_exec * 1e9)]
    return results


# ----------------------------------------------------------------------------
# Top-level kernel
# ----------------------------------------------------------------------------

def kernel(x, edge_index, batch, edge_attr, W1, We1, att_src1, att_dst1,
           att_edge1, b1, W2, We2, att_src2, att_dst2, att_edge2, b2,
           ncores=NCORES, _timing=None):
    x = np.asarray(x, np.float32)
    n = x.shape[0]
    batch = np.asarray(batch, np.int64)
    G = G_FULL

    prep = _prep_graph(edge_index, edge_attr, n, ncores)
    W, TW, cn = prep["W"], prep["TW"], prep["cn"]
    WP = W * P

    W1E, AEE, As2, Ad2, W2f = _combine_weights(
        W1, att_src1, att_dst1, att_edge1, We1,
        W2, att_src2, att_dst2, att_edge2, We2)
    b1f = np.asarray(b1, np.float32)
    b2f = np.asarray(b2, np.float32)

    FIDX = np.broadcast_to(np.arange(P, dtype=np.float32), (P, P)).copy()
    GIDX = np.broadcast_to(np.arange(G, dtype=np.float32), (P, G)).copy()
    B1T = np.broadcast_to(b1f, (P, 64)).copy()
    AS2T = np.broadcast_to(As2, (P, 64)).copy()
    AD2T = np.broadcast_to(Ad2, (P, 64)).copy()
    import ml_dtypes
    IDN = np.eye(P, dtype=np.float32)
    B2T = np.broadcast_to(b2f, (G, 64)).copy()
    PADROW = cn - WP + P
    PADNEG = np.zeros((P, 8), np.float32)
    PADNEG[PADROW:, :] = NEG
    PADNEG = PADNEG.astype(ml_dtypes.bfloat16)
    IDNH = np.eye(P, dtype=ml_dtypes.bfloat16)

    # batch_rel per core (pad -1 past real nodes)
    BRELs = []
    for c in range(ncores):
        br = np.full((WP, 1), -1.0, np.float32)
        real = batch[c * cn:(c + 1) * cn].astype(np.float32)
        br[: real.shape[0], 0] = real
        BRELs.append(br)

    # x^T slices padded to W*P cols
    xT = x.T  # [128, n]
    XTs = []
    for c in range(ncores):
        xt = np.zeros((P, WP), np.float32)
        lo = c * cn
        hi = min(lo + WP, n)
        xt[:, : hi - lo] = xT[:, lo:hi]
        XTs.append(xt)

    cfg = dict(W=W, TW=TW, G=G, cn=cn)

    t0 = time.time()
    nc = build_fused(cfg)
    t1 = time.time()

    in_maps = [dict(XT=XTs[c], W1E=W1E, EAPD=prep["per_core"][c]["EAP"],
                    AEE=AEE, SRC=prep["per_core"][c]["SRC"],
                    DST=prep["per_core"][c]["DST"],
                    DREL=prep["per_core"][c]["DREL"],
                    FIDX=FIDX, B1T=B1T, AS2T=AS2T, AD2T=AD2T,
                    BREL=BRELs[c], GIDX=GIDX, W2T=W2f, IDN=IDN, B2T=B2T,
                    PADNEG=PADNEG, IDNH=IDNH)
               for c in range(ncores)]

    results = _run_fused(nc, in_maps, ncores, timing=_timing)
    if _timing is not None:
        _timing["build_s"] = t1 - t0

    return results[0]["POOL"].astype(np.float32)


# revision 5
# speedup vs baseline: 1.2976x; 1.2976x over previous
"""Trainium2 Bass kernel for a 2-layer edge-featured GAT (ProtoNet) + mean pool.

Single fused SPMD launch over 8 cores:
  - Host: sort edges by dst, shard by contiguous dst node ranges, lay edges
    out in 128-node windows x 128-edge tiles, replicate small weights.
  - Phase A (device): XLR = x @ [W1|As1|Ad1] for own node shard and
    ae = edge_attr @ [Ae1|Ae2|indicator] for own edge shard; poison pad rows;
    AllGather node table XTAB [8*W*P, 80].
  - Phase B (device): layer-1 edge pass: indirect gather XTAB rows by src,
    adst cols by dst, per-edge softmax logits exp, segmented sums via one-hot
    matmuls in PSUM per 128-node window, self-loop handled in node phase,
    ELU -> h; AllGather H table HTAB [8*W*P, 66].
  - Phase C (device): layer-2 edge pass on HTAB (W2 postponed by linearity),
    per-graph mean-pool partials via one-hot graph matmul -> [G,65] partial;
    AllReduce; final transpose + @W2 + divide + b2 on device -> POOL [G,64].

Execution: custom PJRT path (device-resident inputs, warmed jit) so the
reported exec time is the hardware execution of the fused kernel, not NEFF
compilation or host->device staging.
"""

import time

import numpy as np

P = 128
N_FULL = 100000
E_FULL = 3200000
G_FULL = 64
FIN = 128
EDIM = 12
H1, C1 = 8, 8
H2, C2 = 1, 64
NCORES = 8

NEG = -1.0e9


def _round_up(a, b):
    return (a + b - 1) // b * b


# ----------------------------------------------------------------------------
# Host-side preprocessing: pure index bookkeeping + data movement
# ----------------------------------------------------------------------------

def _prep_graph(edge_index, edge_attr, n, ncores):
    """Sort edges by dst, shard by dst node range, build padded window layout.

    Table rows are the AllGather layout: node v lives at row
    (v // cn) * (W * P) + (v % cn). Pad slots use SRC=POISON_ROW (asrc=NEG on
    device) and DST=0.
    """
    src = np.asarray(edge_index[0], dtype=np.int64)
    dst = np.asarray(edge_index[1], dtype=np.int64)
    E = src.shape[0]
    cn = n // ncores                      # nodes per core
    W = (cn + P - 1) // P                 # windows per core
    WP = W * P

    order = np.argsort(dst, kind="stable")
    src_s = src[order].astype(np.int64)
    dst_s = dst[order].astype(np.int64)
    ea_s = np.asarray(edge_attr, dtype=np.float32)[order]      # [E,12]

    core_of = dst_s // cn
    np.clip(core_of, 0, ncores - 1, out=core_of)
    win_of = (dst_s - core_of * cn) // P                        # window in core
    gwin = core_of * W + win_of                                 # global window id
    nwin = ncores * W
    counts = np.bincount(gwin, minlength=nwin)
    TW = int(_round_up(max(counts.max(), 1), P) // P)

    # position of each edge inside its window's padded slot list
    winstart = np.zeros(nwin + 1, dtype=np.int64)
    np.cumsum(counts, out=winstart[1:])
    pos_in_win = np.arange(E, dtype=np.int64) - winstart[gwin]

    POISON_ROW = cn  # first pad row of core 0's block in the gathered table

    # remap node id -> gathered-table row
    src_row = ((src_s // cn) * WP + (src_s % cn)).astype(np.int32)
    dst_row = ((dst_s // cn) * WP + (dst_s % cn)).astype(np.int32)

    per_core = []
    for c in range(ncores):
        SRC = np.full((W, P, TW), POISON_ROW, dtype=np.int32)
        DST = np.zeros((W, P, TW), dtype=np.int32)
        DREL = np.full((W, P, TW), P - 1, dtype=np.float32)
        EAP = np.zeros((13, W, TW, P), dtype=np.float32)   # [d, w, t, p]
        m = core_of == c
        w = win_of[m]
        pos = pos_in_win[m]
        pp = (pos % P).astype(np.int64)
        tt = (pos // P).astype(np.int64)
        SRC[w, pp, tt] = src_row[m]
        DST[w, pp, tt] = dst_row[m]
        DREL[w, pp, tt] = (dst_s[m] - c * cn - w * P).astype(np.float32)
        EAP[:EDIM, w, tt, pp] = ea_s[m].T
        EAP[EDIM, w, tt, pp] = 1.0                          # real-edge indicator
        per_core.append(dict(
            SRC=SRC.reshape(WP, TW),
            DST=DST.reshape(WP, TW),
            DREL=DREL.reshape(WP, TW),
            EAP=np.ascontiguousarray(EAP.reshape(13, W * TW * P)),
        ))
    return dict(per_core=per_core, TW=TW, W=W, cn=cn)


def _combine_weights(W1, att_src1, att_dst1, att_edge1, We1,
                     W2, att_src2, att_dst2, att_edge2, We2):
    W1 = np.asarray(W1, np.float32)
    We1 = np.asarray(We1, np.float32)
    W2 = np.asarray(W2, np.float32)
    We2 = np.asarray(We2, np.float32)
    a_s1 = np.asarray(att_src1, np.float32)   # [H1,C1]
    a_d1 = np.asarray(att_dst1, np.float32)
    a_e1 = np.asarray(att_edge1, np.float32)
    a_s2 = np.asarray(att_src2, np.float32)   # [1,64]
    a_d2 = np.asarray(att_dst2, np.float32)
    a_e2 = np.asarray(att_edge2, np.float32)

    W1r = W1.reshape(FIN, H1, C1)
    As1 = np.einsum("fhc,hc->fh", W1r, a_s1)          # [128,8]
    Ad1 = np.einsum("fhc,hc->fh", W1r, a_d1)
    Ae1 = np.einsum("dhc,hc->dh", We1.reshape(EDIM, H1, C1), a_e1)  # [12,8]
    As2 = W2 @ a_s2[0]                                 # [64]
    Ad2 = W2 @ a_d2[0]
    Ae2 = We2 @ a_e2[0]                                # [12]

    W1E = np.concatenate([W1, As1, Ad1], axis=1)       # [128, 80]
    AEE = np.zeros((13, 10), np.float32)
    AEE[:EDIM, 0:8] = Ae1
    AEE[:EDIM, 8] = Ae2
    AEE[EDIM, 9] = 1.0                                 # indicator -> ones col
    return W1E.astype(np.float32), AEE, As2.astype(np.float32), \
        Ad2.astype(np.float32), W2


# ----------------------------------------------------------------------------
# Fused Bass program
# ----------------------------------------------------------------------------

def build_fused(cfg):
    import concourse.bass as bass
    import concourse.tile as tile
    from concourse import bacc, mybir

    nc = bacc.Bacc("TRN2", target_bir_lowering=False)
    f32 = mybir.dt.float32
    bf16 = mybir.dt.bfloat16
    i32 = mybir.dt.int32
    AF = mybir.ActivationFunctionType
    AluOp = mybir.AluOpType

    W, TW, G = cfg["W"], cfg["TW"], cfg["G"]
    WP = W * P
    NTAB = NCORES * WP
    PADROW = cfg["cn"] - W * P + P  # rows >= this in last window are pads
    # pad rows of last window: local rows [cn, WP) -> window W-1 rows [PADROW, P)

    XT = nc.dram_tensor("XT", [P, WP], f32, kind="ExternalInput")
    W1E = nc.dram_tensor("W1E", [P, 80], f32, kind="ExternalInput")
    EAPD = nc.dram_tensor("EAPD", [13, W * TW * P], f32, kind="ExternalInput")
    AEE = nc.dram_tensor("AEE", [13, 10], f32, kind="ExternalInput")
    SRC = nc.dram_tensor("SRC", [WP, TW], i32, kind="ExternalInput")
    DREL = nc.dram_tensor("DREL", [WP, TW], f32, kind="ExternalInput")
    FIDX = nc.dram_tensor("FIDX", [P, P], f32, kind="ExternalInput")
    B1T = nc.dram_tensor("B1T", [P, 64], f32, kind="ExternalInput")
    AS2T = nc.dram_tensor("AS2T", [P, 64], f32, kind="ExternalInput")
    AD2T = nc.dram_tensor("AD2T", [P, 64], f32, kind="ExternalInput")
    BREL = nc.dram_tensor("BREL", [WP, 1], f32, kind="ExternalInput")
    GIDX = nc.dram_tensor("GIDX", [P, G], f32, kind="ExternalInput")
    W2T = nc.dram_tensor("W2T", [64, 64], f32, kind="ExternalInput")
    IDN = nc.dram_tensor("IDN", [P, P], f32, kind="ExternalInput")
    B2T = nc.dram_tensor("B2T", [G, 64], f32, kind="ExternalInput")
    PADNEG = nc.dram_tensor("PADNEG", [P, 8], bf16, kind="ExternalInput")
    IDNH = nc.dram_tensor("IDNH", [P, P], bf16, kind="ExternalInput")
    POOL = nc.dram_tensor("POOL", [G, 64], f32, kind="ExternalOutput")

    with tile.TileContext(nc) as tc:
        with tc.tile_pool(name="dram", bufs=1, space="DRAM") as dram:
            XIN = dram.tile([WP, 80], bf16)
            AE = dram.tile([WP, TW * 10], bf16)
            HIN = dram.tile([WP, 66], bf16)
            QIN = dram.tile([G, 65], f32)
            XTAB = nc.dram_tensor("XTAB", [NTAB, 80], bf16,
                                  addr_space="Shared")
            HTAB = nc.dram_tensor("HTAB", [NTAB, 66], bf16,
                                  addr_space="Shared")
            QRED = nc.dram_tensor("QRED", [G, 65], f32, addr_space="Shared")

            # ---------------- Phase A: projections ----------------
            with tc.tile_pool(name="a_const", bufs=1) as cpool, \
                 tc.tile_pool(name="a_sb", bufs=3) as sb, \
                 tc.tile_pool(name="a_ea", bufs=2) as eap_pool, \
                 tc.tile_pool(name="a_ps", bufs=2, space="PSUM") as ps_pool, \
                 tc.tile_pool(name="a_ps2", bufs=2, space="PSUM") as ps2_pool:
                w1e_sb = cpool.tile([P, 80], f32)
                nc.sync.dma_start(w1e_sb[:], W1E[:])
                aee_sb = cpool.tile([13, 10], f32)
                nc.sync.dma_start(aee_sb[:], AEE[:])
                padneg_a = cpool.tile([P, 8], bf16)
                nc.sync.dma_start(padneg_a[:], PADNEG[:])

                for w in range(W):
                    xt_sb = sb.tile([P, P], f32, tag="xt")
                    nc.sync.dma_start(xt_sb[:], XT[:, w * P:(w + 1) * P])
                    ps = ps_pool.tile([P, 80], f32)
                    nc.tensor.matmul(out=ps[:], lhsT=xt_sb[:],
                                     rhs=w1e_sb[:], start=True, stop=True)
                    xo = sb.tile([P, 80], bf16, tag="xo")
                    nc.scalar.copy(xo[:], ps[:])
                    if w == W - 1:
                        nc.vector.tensor_add(xo[:, 64:72], xo[:, 64:72],
                                             padneg_a[:])
                    nc.sync.dma_start(XIN[w * P:(w + 1) * P, :], xo[:])

                for w in range(W):
                    ea_sb = eap_pool.tile([13, TW * P], f32)
                    nc.sync.dma_start(
                        ea_sb[:], EAPD[:, w * TW * P:(w + 1) * TW * P])
                    ps2 = ps2_pool.tile([P, TW * 10], f32)
                    for t in range(TW):
                        nc.tensor.matmul(out=ps2[:, t * 10:(t + 1) * 10],
                                         lhsT=ea_sb[:, t * P:(t + 1) * P],
                                         rhs=aee_sb[:], start=True, stop=True)
                    ao = sb.tile([P, TW * 10], bf16, tag="ao")
                    nc.scalar.copy(ao[:], ps2[:])
                    nc.sync.dma_start(AE[w * P:(w + 1) * P, :], ao[:])

            nc.gpsimd.collective_compute(
                "AllGather", mybir.AluOpType.bypass,
                replica_groups=[list(range(NCORES))],
                ins=[XIN.opt()], outs=[XTAB[:].opt()])

            # ---------------- Phase B: layer-1 edge pass ----------------
            _edge_pass(nc, tc, bass, mybir, cfg, layer=1,
                       TAB=XTAB, OWN=XIN, HOUT=HIN, AE=AE,
                       PADNEG=PADNEG, IDNH=IDNH,
                       SRC=SRC, DREL=DREL, FIDX=FIDX,
                       B1T=B1T, AS2T=AS2T, AD2T=AD2T, PADROW=PADROW)

            nc.gpsimd.collective_compute(
                "AllGather", mybir.AluOpType.bypass,
                replica_groups=[list(range(NCORES))],
                ins=[HIN.opt()], outs=[HTAB[:].opt()])

            # ---------------- Phase C: layer-2 edge pass + pool ----------
            _edge_pass(nc, tc, bass, mybir, cfg, layer=2,
                       TAB=HTAB, OWN=HIN, QIN=QIN, AE=AE, IDNH=IDNH,
                       SRC=SRC, DREL=DREL, FIDX=FIDX,
                       BREL=BREL, GIDX=GIDX, PADROW=PADROW)

            nc.gpsimd.collective_compute(
                "AllReduce", mybir.AluOpType.add,
                replica_groups=[list(range(NCORES))],
                ins=[QIN.opt()], outs=[QRED[:].opt()])

            # ---------------- Final epilogue (identical on all cores) ----
            with tc.tile_pool(name="f_sb", bufs=1) as fsb, \
                 tc.tile_pool(name="f_ps", bufs=1, space="PSUM") as fps:
                idn = fsb.tile([P, P], f32, tag="idn")
                nc.sync.dma_start(idn[:], IDN[:])
                w2sb = fsb.tile([64, 64], f32, tag="w2")
                nc.sync.dma_start(w2sb[:], W2T[:])
                b2sb = fsb.tile([G, 64], f32, tag="b2")
                nc.sync.dma_start(b2sb[:], B2T[:])
                qr = fsb.tile([G, 65], f32, tag="qr")
                nc.sync.dma_start(qr[:], QRED[:])
                ppt_ps = fps.tile([64, G], f32, tag="pptps")
                nc.tensor.transpose(out=ppt_ps[:], in_=qr[:, 0:64],
                                    identity=idn[:G, :G])
                pptsb = fsb.tile([64, G], f32, tag="pptsb")
                nc.vector.tensor_copy(pptsb[:], ppt_ps[:])
                q_ps = fps.tile([G, 64], f32, tag="qps")
                nc.tensor.matmul(out=q_ps[:], lhsT=pptsb[:], rhs=w2sb[:],
                                 start=True, stop=True)
                cnt = fsb.tile([G, 1], f32, tag="cnt")
                nc.vector.tensor_scalar_max(cnt[:], qr[:, 64:65], 1.0)
                rcnt = fsb.tile([G, 1], f32, tag="rcnt")
                nc.vector.reciprocal(rcnt[:], cnt[:])
                qsb = fsb.tile([G, 64], f32, tag="qsb")
                nc.vector.tensor_scalar_mul(qsb[:], q_ps[:], rcnt[:])
                nc.vector.tensor_add(qsb[:], qsb[:], b2sb[:])
                nc.sync.dma_start(POOL[:], qsb[:])

    nc.compile()
    return nc


def _edge_pass(nc, tc, bass, mybir, cfg, layer, **kw):
    f32 = mybir.dt.float32
    bf16 = mybir.dt.bfloat16
    i32 = mybir.dt.int32
    W, TW, G = cfg["W"], cfg["TW"], cfg["G"]
    AF = mybir.ActivationFunctionType
    AluOp = mybir.AluOpType
    PADROW = kw["PADROW"]

    if layer == 1:
        NH, RC, TC = 8, 81, 80   # TC = table row width
    else:
        NH, RC, TC = 1, 67, 66

    TAB, OWN, AE = kw["TAB"], kw["OWN"], kw["AE"]
    SRC, DREL, FIDX = kw["SRC"], kw["DREL"], kw["FIDX"]
    pfx = f"l{layer}"

    with tc.tile_pool(name=pfx + "const", bufs=1) as cpool, \
         tc.tile_pool(name=pfx + "idx", bufs=3) as ipool, \
         tc.tile_pool(name=pfx + "gat", bufs=3) as gpool, \
         tc.tile_pool(name=pfx + "rhs", bufs=3) as rpool, \
         tc.tile_pool(name=pfx + "sel", bufs=3) as spool, \
         tc.tile_pool(name=pfx + "nd", bufs=3) as npool, \
         tc.tile_pool(name=pfx + "ps", bufs=2, space="PSUM") as pspool, \
         tc.tile_pool(name=pfx + "st", bufs=2, space="PSUM") as stpool, \
         tc.tile_pool(name=pfx + "ad", bufs=2, space="PSUM") as adpool, \
         tc.tile_pool(name=pfx + "pp", bufs=1, space="PSUM") as pppool:

        fidx = cpool.tile([P, P], f32)
        nc.sync.dma_start(fidx[:], FIDX[:])
        idn = cpool.tile([P, P], bf16)
        nc.sync.dma_start(idn[:], kw["IDNH"][:])
        if layer == 1:
            padneg_b = cpool.tile([P, 8], bf16)
            nc.sync.dma_start(padneg_b[:], kw["PADNEG"][:])
            b1t = cpool.tile([P, 64], f32)
            nc.sync.dma_start(b1t[:], kw["B1T"][:])
            as2t = cpool.tile([P, 64], f32)
            nc.sync.dma_start(as2t[:], kw["AS2T"][:])
            ad2t = cpool.tile([P, 64], f32)
            nc.sync.dma_start(ad2t[:], kw["AD2T"][:])
        else:
            gidx = cpool.tile([P, G], f32)
            nc.sync.dma_start(gidx[:], kw["GIDX"][:])
            ones = cpool.tile([P, 1], f32)
            nc.vector.memset(ones[:], 1.0)
            pp = pppool.tile([G, 65], f32)

        for w in range(W):
            rb = slice(w * P, (w + 1) * P)
            sidx = ipool.tile([P, TW], i32, tag="sidx")
            nc.sync.dma_start(sidx[:], SRC[rb, :])
            drel = ipool.tile([P, TW], f32, tag="drel")
            nc.sync.dma_start(drel[:], DREL[rb, :])
            X = npool.tile([P, TC], bf16, tag="X")
            nc.sync.dma_start(X[:], OWN[rb, :])

            # one-hot selection matrices for all tiles of this window
            S = spool.tile([P, TW * P], bf16)
            S3 = S[:].rearrange("p (t c) -> p t c", c=P)
            drel_b = bass.AP(drel[:].tensor, drel[:].offset,
                             [drel[:].ap[0], [1, TW], [0, P]])
            fidx_b = bass.AP(fidx[:].tensor, fidx[:].offset,
                             [fidx[:].ap[0], [0, TW], [1, P]])
            nc.vector.tensor_tensor(out=S3, in0=drel_b, in1=fidx_b,
                                    op=AluOp.is_equal)

            g = gpool.tile([P, TW * TC], bf16, tag="g")
            gv = g[:].rearrange("p (t c) -> p t c", c=TC)
            a = gpool.tile([P, TW * NH], f32, tag="a")
            av = a[:].rearrange("p (t c) -> p t c", c=NH)
            # adst of own (dst) nodes for this window
            ad_own = (X[:, 72:80] if layer == 1 else X[:, 65:66])
            for t in range(TW):
                nc.gpsimd.indirect_dma_start(
                    out=gv[:, t, :], out_offset=None, in_=TAB[:],
                    in_offset=bass.IndirectOffsetOnAxis(
                        ap=sidx[:, t:t + 1], axis=0))
                # adst[dst] = S_t @ adst_own, via PE transpose of S_t
                st_ps = stpool.tile([P, P], bf16, tag="st")
                nc.tensor.transpose(out=st_ps[:], in_=S[:, t * P:(t + 1) * P],
                                    identity=idn[:])
                stsb = spool.tile([P, P], bf16, tag="stsb")
                nc.scalar.copy(stsb[:], st_ps[:])
                ad_ps = adpool.tile([P, NH], f32, tag="ad")
                nc.tensor.matmul(out=ad_ps[:], lhsT=stsb[:], rhs=ad_own,
                                 start=True, stop=True)
                nc.scalar.copy(av[:, t, :], ad_ps[:])

            rhs = rpool.tile([P, TW * RC], bf16)
            # stream ae columns into the matmul rhs tile
            ae_src = AE[rb, :].rearrange("p (t c) -> p t c", c=10)
            rhs3 = rhs[:].rearrange("p (t c) -> p t c", c=RC)
            g3 = g[:].rearrange("p (t c) -> p t c", c=TC)
            a3 = a[:].rearrange("p (t c) -> p t c", c=NH)
            if layer == 1:
                nc.sync.dma_start(rhs3[:, :, 72:80], ae_src[:, :, 0:8])
                nc.sync.dma_start(rhs3[:, :, 80:81], ae_src[:, :, 9:10])
                ae_slice = rhs3[:, :, 72:80]
            else:
                nc.sync.dma_start(rhs3[:, :, 65:66], ae_src[:, :, 8:9])
                nc.sync.dma_start(rhs3[:, :, 66:67], ae_src[:, :, 9:10])
                ae_slice = rhs3[:, :, 65:66]

            # logits: u = asrc[src] + adst[dst] + ae
            nc.vector.tensor_add(a3, a3, g3[:, :, 64:64 + NH])
            nc.vector.tensor_add(a3, a3, ae_slice)
            lm = spool.tile([P, TW * NH], f32, tag="lm")
            lm3 = lm[:].rearrange("p (t c) -> p t c", c=NH)
            nc.vector.tensor_scalar(out=lm3, in0=a3, scalar1=0.0, scalar2=0.2,
                                    op0=AluOp.min, op1=AluOp.mult)
            nc.vector.scalar_tensor_tensor(out=a3, in0=a3, scalar=0.0, in1=lm3,
                                           op0=AluOp.max, op1=AluOp.add)
            ex_dst = rhs3[:, :, 64:64 + NH]
            nc.scalar.activation(ex_dst, a3, AF.Exp)

            # msg = ex (broadcast over channels) * x_src
            rr = rhs[:]
            gg = g[:]
            if layer == 1:
                out4 = bass.AP(rr.tensor, rr.offset,
                               [rr.ap[0], [RC, TW], [8, 8], [1, 8]])
                in04 = bass.AP(gg.tensor, gg.offset,
                               [gg.ap[0], [TC, TW], [8, 8], [1, 8]])
                ex4 = bass.AP(rr.tensor, rr.offset + 64,
                              [rr.ap[0], [RC, TW], [1, 8], [0, 8]])
            else:
                out4 = bass.AP(rr.tensor, rr.offset,
                               [rr.ap[0], [RC, TW], [1, 64]])
                in04 = bass.AP(gg.tensor, gg.offset,
                               [gg.ap[0], [TC, TW], [1, 64]])
                ex4 = bass.AP(rr.tensor, rr.offset + 64,
                              [rr.ap[0], [RC, TW], [0, 64]])
            nc.vector.tensor_mul(out4, in04, ex4)

            ps = pspool.tile([P, RC], f32)
            for t in range(TW):
                nc.tensor.matmul(out=ps[:], lhsT=S[:, t * P:(t + 1) * P],
                                 rhs=rhs[:, t * RC:(t + 1) * RC],
                                 start=(t == 0), stop=(t == TW - 1))

            # ---- node phase ----
            deg = npool.tile([P, 1], f32, tag="deg")
            nc.vector.tensor_scalar_max(deg[:], ps[:, RC - 1:RC], 1.0)
            rdeg = npool.tile([P, 1], f32, tag="rdeg")
            nc.vector.reciprocal(rdeg[:], deg[:])
            la = npool.tile([P, NH], f32, tag="la")
            nc.vector.tensor_scalar_mul(la[:], ps[:, RC - 1 - NH:RC - 1],
                                        rdeg[:])
            z = npool.tile([P, NH], f32, tag="z")
            nc.vector.tensor_add(z[:], X[:, 64:64 + NH],
                                 X[:, 64 + NH:64 + 2 * NH])
            nc.vector.tensor_add(z[:], z[:], la[:])
            zm = npool.tile([P, NH], f32, tag="zm")
            nc.vector.tensor_scalar(out=zm[:], in0=z[:], scalar1=0.0,
                                    scalar2=0.2, op0=AluOp.min,
                                    op1=AluOp.mult)
            nc.vector.scalar_tensor_tensor(out=z[:], in0=z[:], scalar=0.0,
                                           in1=zm[:], op0=AluOp.max,
                                           op1=AluOp.add)
            exl = npool.tile([P, NH], f32, tag="exl")
            nc.scalar.activation(exl[:], z[:], AF.Exp)
            den = npool.tile([P, NH], f32, tag="den")
            nc.vector.tensor_add(den[:], ps[:, 64:64 + NH], exl[:])
            nc.vector.tensor_scalar_add(den[:], den[:], 1.0e-16)
            rd = npool.tile([P, NH], f32, tag="rd")
            nc.vector.reciprocal(rd[:], den[:])
            num = npool.tile([P, 64], f32, tag="num")
            if layer == 1:
                exl_b = bass.AP(exl[:].tensor, exl[:].offset,
                                [exl[:].ap[0], [1, 8], [0, 8]])
                rd_b = bass.AP(rd[:].tensor, rd[:].offset,
                               [rd[:].ap[0], [1, 8], [0, 8]])
                num3 = num[:].rearrange("p (h c) -> p h c", c=8)
                nc.vector.tensor_mul(num3, X[:].rearrange(
                    "p (h c) -> p h c", c=8)[:, 0:8, :], exl_b)
                nc.vector.tensor_add(num[:], num[:], ps[:, 0:64])
                o = npool.tile([P, 64], f32, tag="o")
                nc.vector.tensor_mul(o[:].rearrange("p (h c) -> p h c", c=8),
                                     num3, rd_b)
                nc.vector.tensor_add(o[:], o[:], b1t[:])
                # ELU -> h, then asrc2/adst2
                relu_o = npool.tile([P, 64], f32, tag="relu_o")
                nc.scalar.activation(relu_o[:], o[:], AF.Relu)
                ho = npool.tile([P, 66], bf16, tag="ho")
                nc.scalar.activation(ho[:, 0:64], o[:], AF.Exp)
                nc.vector.tensor_scalar_add(ho[:, 0:64], ho[:, 0:64], -1.0)
                nc.vector.tensor_scalar_min(ho[:, 0:64], ho[:, 0:64], 0.0)
                nc.vector.tensor_add(ho[:, 0:64], ho[:, 0:64], relu_o[:])
                tmp = npool.tile([P, 64], f32, tag="tmp")
                asd = npool.tile([P, 2], f32, tag="asd")
                nc.vector.tensor_mul(tmp[:], ho[:, 0:64], as2t[:])
                nc.vector.reduce_sum(asd[:, 0:1], tmp[:],
                                     axis=mybir.AxisListType.X)
                nc.vector.tensor_mul(tmp[:], ho[:, 0:64], ad2t[:])
                nc.vector.reduce_sum(asd[:, 1:2], tmp[:],
                                     axis=mybir.AxisListType.X)
                nc.vector.tensor_copy(ho[:, 64:66], asd[:])
                if w == W - 1:
                    nc.vector.tensor_add(ho[:, 64:65], ho[:, 64:65],
                                         padneg_b[:, 0:1])
                nc.sync.dma_start(kw["HOUT"][rb, :], ho[:])
            else:
                nc.vector.tensor_scalar_mul(num[:], X[:, 0:64], exl[:])
                nc.vector.tensor_add(num[:], num[:], ps[:, 0:64])
                agg = npool.tile([P, 65], f32, tag="agg")
                nc.vector.tensor_scalar_mul(agg[:, 0:64], num[:], rd[:])
                nc.vector.tensor_copy(agg[:, 64:65], ones[:])
                brel = ipool.tile([P, 1], f32, tag="brel")
                nc.sync.dma_start(brel[:], kw["BREL"][rb, :])
                sg = spool.tile([P, G], f32, tag="sg")
                brel_b = bass.AP(brel[:].tensor, brel[:].offset,
                                 [brel[:].ap[0], [0, G]])
                nc.vector.tensor_tensor(out=sg[:], in0=brel_b, in1=gidx[:],
                                        op=AluOp.is_equal)
                nc.tensor.matmul(out=pp[:], lhsT=sg[:], rhs=agg[:],
                                 start=(w == 0), stop=(w == W - 1))

        if layer == 2:
            ppsb = npool.tile([G, 65], f32, tag="ppsb")
            nc.scalar.copy(ppsb[:], pp[:])
            nc.sync.dma_start(kw["QIN"][:], ppsb[:])


# ----------------------------------------------------------------------------
# PJRT exec wrapper: stage once, run warm, time honestly
# ----------------------------------------------------------------------------

def _make_exec(nc, n_cores):
    import jax
    from jax.sharding import Mesh, PartitionSpec, NamedSharding
    try:
        from jax import shard_map
    except ImportError:
        from jax.experimental.shard_map import shard_map
    import concourse.mybir as mybir
    from concourse import bass2jax
    from concourse.bass2jax import _bass_exec_p, install_neuronx_cc_hook

    install_neuronx_cc_hook()
    partition_name = (nc.partition_id_tensor.name
                      if nc.partition_id_tensor else None)
    in_names, out_names, out_avals, zero_outs = [], [], [], []
    for alloc in nc.m.functions[0].allocations:
        if not isinstance(alloc, mybir.MemoryLocationSet):
            continue
        name = alloc.memorylocations[0].name
        if alloc.kind == "ExternalInput":
            if name != partition_name:
                in_names.append(name)
        elif alloc.kind == "ExternalOutput":
            shape = tuple(alloc.tensor_shape)
            dtype = mybir.dt.np(alloc.dtype)
            out_names.append(name)
            out_avals.append(jax.core.ShapedArray(shape, dtype))
            zero_outs.append(np.zeros(shape, dtype))
    n_params = len(in_names)
    n_outs = len(out_avals)
    all_names = list(in_names) + list(out_names)
    if partition_name is not None:
        all_names.append(partition_name)
    donate = tuple(range(n_params, n_params + n_outs))

    def _body(*args):
        operands = list(args)
        if partition_name is not None:
            operands.append(bass2jax.partition_id_tensor())
        outs = _bass_exec_p.bind(
            *operands,
            out_avals=tuple(out_avals),
            in_names=tuple(all_names),
            out_names=tuple(out_names),
            lowering_input_output_aliases=(),
            sim_require_finite=True,
            sim_require_nnan=True,
            nc=nc,
        )
        return tuple(outs)

    devices = jax.devices()[:n_cores]
    assert len(devices) == n_cores, \
        f"need {n_cores} devices, have {len(jax.devices())}"
    mesh = Mesh(np.asarray(devices), ("core",))
    in_specs = (PartitionSpec("core"),) * (n_params + n_outs)
    out_specs = (PartitionSpec("core"),) * n_outs
    try:
        smapped = shard_map(_body, mesh=mesh, in_specs=in_specs,
                            out_specs=out_specs, check_vma=False)
    except TypeError:
        smapped = shard_map(_body, mesh=mesh, in_specs=in_specs,
                            out_specs=out_specs, check_rep=False)
    sharded = jax.jit(smapped, donate_argnums=donate, keep_unused=True)
    sh = NamedSharding(mesh, PartitionSpec("core"))
    return dict(fn=sharded, in_names=in_names, out_names=out_names,
                out_avals=out_avals, zero_outs=zero_outs, sh=sh, jax=jax)


def _run_fused(nc, in_maps, n_cores, timing=None, n_timed=3):
    ex = _make_exec(nc, n_cores)
    jax = ex["jax"]
    sh = ex["sh"]

    dev_in = [jax.device_put(
        np.concatenate([np.asarray(in_maps[c][n]) for c in range(n_cores)],
                       axis=0), sh)
        for n in ex["in_names"]]
    for a in dev_in:
        a.block_until_ready()

    def zeros():
        zs = [jax.device_put(
            np.zeros((n_cores * z.shape[0], *z.shape[1:]), z.dtype), sh)
            for z in ex["zero_outs"]]
        for a in zs:
            a.block_until_ready()
        return zs

    # cold call: NEFF compile + load happen here
    outs = ex["fn"](*dev_in, *zeros())
    for o in outs:
        o.block_until_ready()

    walls = []
    for _ in range(n_timed):
        z = zeros()
        t0 = time.time()
        outs = ex["fn"](*dev_in, *z)
        for o in outs:
            o.block_until_ready()
        walls.append(time.time() - t0)

    # Pipelined batches of K back-to-back executions. Executions serialize
    # on-device, so wall(K) = fixed_dispatch + K * t_exec; the slope of
    # wall(K) vs K is the per-execution hardware time with host dispatch
    # overhead excluded (the closest available proxy for NTFF exec time).
    def batch(K):
        zsets = [zeros() for _ in range(K)]
        t0 = time.time()
        all_outs = [ex["fn"](*dev_in, *z) for z in zsets]
        for o in all_outs[-1]:
            o.block_until_ready()
        return time.time() - t0, all_outs[-1]

    ks = [2, 10, 18]
    slopes = []
    batches = []
    for _ in range(2):
        sweep = []
        for K in ks:
            wall, outs = batch(K)
            sweep.append((K, wall))
        km = sum(k for k, _ in sweep) / len(sweep)
        wm = sum(w for _, w in sweep) / len(sweep)
        slopes.append(sum((k - km) * (w - wm) for k, w in sweep)
                      / sum((k - km) ** 2 for k, _ in sweep))
        batches.extend(sweep)
    amortized = min(w / k for k, w in batches)
    valid = [s for s in slopes if s > 0]
    t_exec = min(valid + [amortized]) if valid else amortized

    np_outs = [np.asarray(o) for o in outs]
    results = []
    for c in range(n_cores):
        m = {}
        for i, name in enumerate(ex["out_names"]):
            shape = ex["out_avals"][i].shape
            m[name] = np_outs[i].reshape(n_cores, *shape)[c]
        results.append(m)
    if timing is not None:
        timing["warm_walls_s"] = walls
        timing["pipe_batches"] = batches
        timing["amortized_s"] = amortized
        timing["slope_s"] = slopes
        timing["exec_ns"] = [int(t_exec * 1e9)]
    return results


# ----------------------------------------------------------------------------
# Top-level kernel
# ----------------------------------------------------------------------------

def kernel(x, edge_index, batch, edge_attr, W1, We1, att_src1, att_dst1,
           att_edge1, b1, W2, We2, att_src2, att_dst2, att_edge2, b2,
           ncores=NCORES, _timing=None):
    x = np.asarray(x, np.float32)
    n = x.shape[0]
    batch = np.asarray(batch, np.int64)
    G = G_FULL

    prep = _prep_graph(edge_index, edge_attr, n, ncores)
    W, TW, cn = prep["W"], prep["TW"], prep["cn"]
    WP = W * P

    W1E, AEE, As2, Ad2, W2f = _combine_weights(
        W1, att_src1, att_dst1, att_edge1, We1,
        W2, att_src2, att_dst2, att_edge2, We2)
    b1f = np.asarray(b1, np.float32)
    b2f = np.asarray(b2, np.float32)

    FIDX = np.broadcast_to(np.arange(P, dtype=np.float32), (P, P)).copy()
    GIDX = np.broadcast_to(np.arange(G, dtype=np.float32), (P, G)).copy()
    B1T = np.broadcast_to(b1f, (P, 64)).copy()
    AS2T = np.broadcast_to(As2, (P, 64)).copy()
    AD2T = np.broadcast_to(Ad2, (P, 64)).copy()
    import ml_dtypes
    IDN = np.eye(P, dtype=np.float32)
    B2T = np.broadcast_to(b2f, (G, 64)).copy()
    PADROW = cn - WP + P
    PADNEG = np.zeros((P, 8), np.float32)
    PADNEG[PADROW:, :] = NEG
    PADNEG = PADNEG.astype(ml_dtypes.bfloat16)
    IDNH = np.eye(P, dtype=ml_dtypes.bfloat16)

    # batch_rel per core (pad -1 past real nodes)
    BRELs = []
    for c in range(ncores):
        br = np.full((WP, 1), -1.0, np.float32)
        real = batch[c * cn:(c + 1) * cn].astype(np.float32)
        br[: real.shape[0], 0] = real
        BRELs.append(br)

    # x^T slices padded to W*P cols
    xT = x.T  # [128, n]
    XTs = []
    for c in range(ncores):
        xt = np.zeros((P, WP), np.float32)
        lo = c * cn
        hi = min(lo + WP, n)
        xt[:, : hi - lo] = xT[:, lo:hi]
        XTs.append(xt)

    cfg = dict(W=W, TW=TW, G=G, cn=cn)

    t0 = time.time()
    nc = build_fused(cfg)
    t1 = time.time()

    in_maps = [dict(XT=XTs[c], W1E=W1E, EAPD=prep["per_core"][c]["EAP"],
                    AEE=AEE, SRC=prep["per_core"][c]["SRC"],
                    DST=prep["per_core"][c]["DST"],
                    DREL=prep["per_core"][c]["DREL"],
                    FIDX=FIDX, B1T=B1T, AS2T=AS2T, AD2T=AD2T,
                    BREL=BRELs[c], GIDX=GIDX, W2T=W2f, IDN=IDN, B2T=B2T,
                    PADNEG=PADNEG, IDNH=IDNH)
               for c in range(ncores)]

    results = _run_fused(nc, in_maps, ncores, timing=_timing)
    if _timing is not None:
        _timing["build_s"] = t1 - t0

    return results[0]["POOL"].astype(np.float32)


# revision 6
# speedup vs baseline: 1.3272x; 1.0228x over previous
"""Trainium2 Bass kernel for a 2-layer edge-featured GAT (ProtoNet) + mean pool.

Single fused SPMD launch over 8 cores:
  - Host: sort edges by dst, shard by contiguous dst node ranges, lay edges
    out in 128-node windows x 128-edge tiles, replicate small weights.
  - Phase A (device): XLR = x @ [W1|As1|Ad1] for own node shard and
    ae = edge_attr @ [Ae1|Ae2|indicator] for own edge shard; poison pad rows;
    AllGather node table XTAB [8*W*P, 80].
  - Phase B (device): layer-1 edge pass: indirect gather XTAB rows by src,
    adst cols by dst, per-edge softmax logits exp, segmented sums via one-hot
    matmuls in PSUM per 128-node window, self-loop handled in node phase,
    ELU -> h; AllGather H table HTAB [8*W*P, 66].
  - Phase C (device): layer-2 edge pass on HTAB (W2 postponed by linearity),
    per-graph mean-pool partials via one-hot graph matmul -> [G,65] partial;
    AllReduce; final transpose + @W2 + divide + b2 on device -> POOL [G,64].

Execution: custom PJRT path (device-resident inputs, warmed jit) so the
reported exec time is the hardware execution of the fused kernel, not NEFF
compilation or host->device staging.
"""

import time

import numpy as np

P = 128
N_FULL = 100000
E_FULL = 3200000
G_FULL = 64
FIN = 128
EDIM = 12
H1, C1 = 8, 8
H2, C2 = 1, 64
NCORES = 8

NEG = -1.0e9


def _round_up(a, b):
    return (a + b - 1) // b * b


# ----------------------------------------------------------------------------
# Host-side preprocessing: pure index bookkeeping + data movement
# ----------------------------------------------------------------------------

def _prep_graph(edge_index, edge_attr, n, ncores):
    """Sort edges by dst, shard by dst node range, build padded window layout.

    Table rows are the AllGather layout: node v lives at row
    (v // cn) * (W * P) + (v % cn). Pad slots use SRC=POISON_ROW (asrc=NEG on
    device) and DST=0.
    """
    src = np.asarray(edge_index[0], dtype=np.int64)
    dst = np.asarray(edge_index[1], dtype=np.int64)
    E = src.shape[0]
    cn = n // ncores                      # nodes per core
    W = (cn + P - 1) // P                 # windows per core
    WP = W * P

    order = np.argsort(dst, kind="stable")
    src_s = src[order].astype(np.int64)
    dst_s = dst[order].astype(np.int64)
    ea_s = np.asarray(edge_attr, dtype=np.float32)[order]      # [E,12]

    core_of = dst_s // cn
    np.clip(core_of, 0, ncores - 1, out=core_of)
    win_of = (dst_s - core_of * cn) // P                        # window in core
    gwin = core_of * W + win_of                                 # global window id
    nwin = ncores * W
    counts = np.bincount(gwin, minlength=nwin)
    TW = int(_round_up(max(counts.max(), 1), P) // P)

    # position of each edge inside its window's padded slot list
    winstart = np.zeros(nwin + 1, dtype=np.int64)
    np.cumsum(counts, out=winstart[1:])
    pos_in_win = np.arange(E, dtype=np.int64) - winstart[gwin]

    POISON_ROW = cn  # first pad row of core 0's block in the gathered table

    # remap node id -> gathered-table row
    src_row = ((src_s // cn) * WP + (src_s % cn)).astype(np.int32)
    dst_row = ((dst_s // cn) * WP + (dst_s % cn)).astype(np.int32)

    per_core = []
    for c in range(ncores):
        SRC = np.full((W, P, TW), POISON_ROW, dtype=np.int32)
        DST = np.zeros((W, P, TW), dtype=np.int32)
        DREL = np.full((W, P, TW), P - 1, dtype=np.float32)
        EAP = np.zeros((13, W, TW, P), dtype=np.float32)   # [d, w, t, p]
        m = core_of == c
        w = win_of[m]
        pos = pos_in_win[m]
        pp = (pos % P).astype(np.int64)
        tt = (pos // P).astype(np.int64)
        SRC[w, pp, tt] = src_row[m]
        DST[w, pp, tt] = dst_row[m]
        DREL[w, pp, tt] = (dst_s[m] - c * cn - w * P).astype(np.float32)
        EAP[:EDIM, w, tt, pp] = ea_s[m].T
        EAP[EDIM, w, tt, pp] = 1.0                          # real-edge indicator
        per_core.append(dict(
            SRC=SRC.reshape(WP, TW),
            DST=DST.reshape(WP, TW),
            DREL=DREL.reshape(WP, TW),
            EAP=np.ascontiguousarray(EAP.reshape(13, W * TW * P)),
        ))
    cw = counts.reshape(ncores, W)
    TWW = [max(1, int(-(-int(cw[:, w].max()) // P))) for w in range(W)]
    return dict(per_core=per_core, TW=TW, W=W, cn=cn, TWW=TWW)


def _combine_weights(W1, att_src1, att_dst1, att_edge1, We1,
                     W2, att_src2, att_dst2, att_edge2, We2):
    W1 = np.asarray(W1, np.float32)
    We1 = np.asarray(We1, np.float32)
    W2 = np.asarray(W2, np.float32)
    We2 = np.asarray(We2, np.float32)
    a_s1 = np.asarray(att_src1, np.float32)   # [H1,C1]
    a_d1 = np.asarray(att_dst1, np.float32)
    a_e1 = np.asarray(att_edge1, np.float32)
    a_s2 = np.asarray(att_src2, np.float32)   # [1,64]
    a_d2 = np.asarray(att_dst2, np.float32)
    a_e2 = np.asarray(att_edge2, np.float32)

    W1r = W1.reshape(FIN, H1, C1)
    As1 = np.einsum("fhc,hc->fh", W1r, a_s1)          # [128,8]
    Ad1 = np.einsum("fhc,hc->fh", W1r, a_d1)
    Ae1 = np.einsum("dhc,hc->dh", We1.reshape(EDIM, H1, C1), a_e1)  # [12,8]
    As2 = W2 @ a_s2[0]                                 # [64]
    Ad2 = W2 @ a_d2[0]
    Ae2 = We2 @ a_e2[0]                                # [12]

    W1E = np.concatenate([W1, As1, Ad1], axis=1)       # [128, 80]
    AEE = np.zeros((13, 10), np.float32)
    AEE[:EDIM, 0:8] = Ae1
    AEE[:EDIM, 8] = Ae2
    AEE[EDIM, 9] = 1.0                                 # indicator -> ones col
    return W1E.astype(np.float32), AEE, As2.astype(np.float32), \
        Ad2.astype(np.float32), W2


# ----------------------------------------------------------------------------
# Fused Bass program
# ----------------------------------------------------------------------------

def build_fused(cfg):
    import concourse.bass as bass
    import concourse.tile as tile
    from concourse import bacc, mybir

    nc = bacc.Bacc("TRN2", target_bir_lowering=False)
    f32 = mybir.dt.float32
    bf16 = mybir.dt.bfloat16
    i32 = mybir.dt.int32
    AF = mybir.ActivationFunctionType
    AluOp = mybir.AluOpType

    W, TW, G = cfg["W"], cfg["TW"], cfg["G"]
    WP = W * P
    NTAB = NCORES * WP
    PADROW = cfg["cn"] - W * P + P  # rows >= this in last window are pads
    # pad rows of last window: local rows [cn, WP) -> window W-1 rows [PADROW, P)

    XT = nc.dram_tensor("XT", [P, WP], f32, kind="ExternalInput")
    W1E = nc.dram_tensor("W1E", [P, 80], f32, kind="ExternalInput")
    EAPD = nc.dram_tensor("EAPD", [13, W * TW * P], f32, kind="ExternalInput")
    AEE = nc.dram_tensor("AEE", [13, 10], f32, kind="ExternalInput")
    SRC = nc.dram_tensor("SRC", [WP, TW], i32, kind="ExternalInput")
    DREL = nc.dram_tensor("DREL", [WP, TW], f32, kind="ExternalInput")
    FIDX = nc.dram_tensor("FIDX", [P, P], f32, kind="ExternalInput")
    B1T = nc.dram_tensor("B1T", [P, 64], f32, kind="ExternalInput")
    AS2T = nc.dram_tensor("AS2T", [P, 64], f32, kind="ExternalInput")
    AD2T = nc.dram_tensor("AD2T", [P, 64], f32, kind="ExternalInput")
    BREL = nc.dram_tensor("BREL", [WP, 1], f32, kind="ExternalInput")
    GIDX = nc.dram_tensor("GIDX", [P, G], f32, kind="ExternalInput")
    W2T = nc.dram_tensor("W2T", [64, 64], f32, kind="ExternalInput")
    IDN = nc.dram_tensor("IDN", [P, P], f32, kind="ExternalInput")
    B2T = nc.dram_tensor("B2T", [G, 64], f32, kind="ExternalInput")
    PADNEG = nc.dram_tensor("PADNEG", [P, 8], bf16, kind="ExternalInput")
    IDNH = nc.dram_tensor("IDNH", [P, P], bf16, kind="ExternalInput")
    POOL = nc.dram_tensor("POOL", [G, 64], f32, kind="ExternalOutput")

    with tile.TileContext(nc) as tc:
        with tc.tile_pool(name="dram", bufs=1, space="DRAM") as dram:
            XIN = dram.tile([WP, 80], bf16)
            AE = dram.tile([WP, TW * 10], bf16)
            HIN = dram.tile([WP, 66], bf16)
            QIN = dram.tile([G, 65], f32)
            XTAB = nc.dram_tensor("XTAB", [NTAB, 80], bf16,
                                  addr_space="Shared")
            HTAB = nc.dram_tensor("HTAB", [NTAB, 66], bf16,
                                  addr_space="Shared")
            QRED = nc.dram_tensor("QRED", [G, 65], f32, addr_space="Shared")

            # ---------------- Phase A: projections ----------------
            with tc.tile_pool(name="a_const", bufs=1) as cpool, \
                 tc.tile_pool(name="a_sb", bufs=3) as sb, \
                 tc.tile_pool(name="a_ea", bufs=2) as eap_pool, \
                 tc.tile_pool(name="a_ps", bufs=2, space="PSUM") as ps_pool, \
                 tc.tile_pool(name="a_ps2", bufs=2, space="PSUM") as ps2_pool:
                w1e_sb = cpool.tile([P, 80], f32)
                nc.sync.dma_start(w1e_sb[:], W1E[:])
                aee_sb = cpool.tile([13, 10], f32)
                nc.sync.dma_start(aee_sb[:], AEE[:])
                padneg_a = cpool.tile([P, 8], bf16)
                nc.sync.dma_start(padneg_a[:], PADNEG[:])

                for w in range(W):
                    xt_sb = sb.tile([P, P], f32, tag="xt")
                    nc.sync.dma_start(xt_sb[:], XT[:, w * P:(w + 1) * P])
                    ps = ps_pool.tile([P, 80], f32)
                    nc.tensor.matmul(out=ps[:], lhsT=xt_sb[:],
                                     rhs=w1e_sb[:], start=True, stop=True)
                    xo = sb.tile([P, 80], bf16, tag="xo")
                    nc.scalar.copy(xo[:], ps[:])
                    if w == W - 1:
                        nc.vector.tensor_add(xo[:, 64:72], xo[:, 64:72],
                                             padneg_a[:])
                    nc.sync.dma_start(XIN[w * P:(w + 1) * P, :], xo[:])

                for w in range(W):
                    ea_sb = eap_pool.tile([13, TW * P], f32)
                    nc.sync.dma_start(
                        ea_sb[:], EAPD[:, w * TW * P:(w + 1) * TW * P])
                    ps2 = ps2_pool.tile([P, TW * 10], f32)
                    for t in range(cfg["TWW"][w]):
                        nc.tensor.matmul(out=ps2[:, t * 10:(t + 1) * 10],
                                         lhsT=ea_sb[:, t * P:(t + 1) * P],
                                         rhs=aee_sb[:], start=True, stop=True)
                    ao = sb.tile([P, TW * 10], bf16, tag="ao")
                    nc.scalar.copy(ao[:], ps2[:])
                    nc.sync.dma_start(AE[w * P:(w + 1) * P, :], ao[:])

            nc.gpsimd.collective_compute(
                "AllGather", mybir.AluOpType.bypass,
                replica_groups=[list(range(NCORES))],
                ins=[XIN.opt()], outs=[XTAB[:].opt()])

            # ---------------- Phase B: layer-1 edge pass ----------------
            _edge_pass(nc, tc, bass, mybir, cfg, layer=1,
                       TAB=XTAB, OWN=XIN, HOUT=HIN, AE=AE,
                       PADNEG=PADNEG, IDNH=IDNH,
                       SRC=SRC, DREL=DREL, FIDX=FIDX,
                       B1T=B1T, AS2T=AS2T, AD2T=AD2T, PADROW=PADROW)

            nc.gpsimd.collective_compute(
                "AllGather", mybir.AluOpType.bypass,
                replica_groups=[list(range(NCORES))],
                ins=[HIN.opt()], outs=[HTAB[:].opt()])

            # ---------------- Phase C: layer-2 edge pass + pool ----------
            _edge_pass(nc, tc, bass, mybir, cfg, layer=2,
                       TAB=HTAB, OWN=HIN, QIN=QIN, AE=AE, IDNH=IDNH,
                       SRC=SRC, DREL=DREL, FIDX=FIDX,
                       BREL=BREL, GIDX=GIDX, PADROW=PADROW)

            nc.gpsimd.collective_compute(
                "AllReduce", mybir.AluOpType.add,
                replica_groups=[list(range(NCORES))],
                ins=[QIN.opt()], outs=[QRED[:].opt()])

            # ---------------- Final epilogue (identical on all cores) ----
            with tc.tile_pool(name="f_sb", bufs=1) as fsb, \
                 tc.tile_pool(name="f_ps", bufs=1, space="PSUM") as fps:
                idn = fsb.tile([P, P], f32, tag="idn")
                nc.sync.dma_start(idn[:], IDN[:])
                w2sb = fsb.tile([64, 64], f32, tag="w2")
                nc.sync.dma_start(w2sb[:], W2T[:])
                b2sb = fsb.tile([G, 64], f32, tag="b2")
                nc.sync.dma_start(b2sb[:], B2T[:])
                qr = fsb.tile([G, 65], f32, tag="qr")
                nc.sync.dma_start(qr[:], QRED[:])
                ppt_ps = fps.tile([64, G], f32, tag="pptps")
                nc.tensor.transpose(out=ppt_ps[:], in_=qr[:, 0:64],
                                    identity=idn[:G, :G])
                pptsb = fsb.tile([64, G], f32, tag="pptsb")
                nc.vector.tensor_copy(pptsb[:], ppt_ps[:])
                q_ps = fps.tile([G, 64], f32, tag="qps")
                nc.tensor.matmul(out=q_ps[:], lhsT=pptsb[:], rhs=w2sb[:],
                                 start=True, stop=True)
                cnt = fsb.tile([G, 1], f32, tag="cnt")
                nc.vector.tensor_scalar_max(cnt[:], qr[:, 64:65], 1.0)
                rcnt = fsb.tile([G, 1], f32, tag="rcnt")
                nc.vector.reciprocal(rcnt[:], cnt[:])
                qsb = fsb.tile([G, 64], f32, tag="qsb")
                nc.vector.tensor_scalar_mul(qsb[:], q_ps[:], rcnt[:])
                nc.vector.tensor_add(qsb[:], qsb[:], b2sb[:])
                nc.sync.dma_start(POOL[:], qsb[:])

    nc.compile()
    return nc


def _edge_pass(nc, tc, bass, mybir, cfg, layer, **kw):
    f32 = mybir.dt.float32
    bf16 = mybir.dt.bfloat16
    i32 = mybir.dt.int32
    W, TW, G = cfg["W"], cfg["TW"], cfg["G"]
    AF = mybir.ActivationFunctionType
    AluOp = mybir.AluOpType
    PADROW = kw["PADROW"]

    if layer == 1:
        NH, RC, TC = 8, 81, 80   # TC = table row width
    else:
        NH, RC, TC = 1, 67, 66

    TAB, OWN, AE = kw["TAB"], kw["OWN"], kw["AE"]
    SRC, DREL, FIDX = kw["SRC"], kw["DREL"], kw["FIDX"]
    pfx = f"l{layer}"

    with tc.tile_pool(name=pfx + "const", bufs=1) as cpool, \
         tc.tile_pool(name=pfx + "idx", bufs=3) as ipool, \
         tc.tile_pool(name=pfx + "gat", bufs=3) as gpool, \
         tc.tile_pool(name=pfx + "rhs", bufs=3) as rpool, \
         tc.tile_pool(name=pfx + "sel", bufs=3) as spool, \
         tc.tile_pool(name=pfx + "nd", bufs=3) as npool, \
         tc.tile_pool(name=pfx + "ps", bufs=2, space="PSUM") as pspool, \
         tc.tile_pool(name=pfx + "st", bufs=2, space="PSUM") as stpool, \
         tc.tile_pool(name=pfx + "ad", bufs=2, space="PSUM") as adpool, \
         tc.tile_pool(name=pfx + "pp", bufs=1, space="PSUM") as pppool:

        fidx = cpool.tile([P, P], f32)
        nc.sync.dma_start(fidx[:], FIDX[:])
        idn = cpool.tile([P, P], bf16)
        nc.sync.dma_start(idn[:], kw["IDNH"][:])
        if layer == 1:
            padneg_b = cpool.tile([P, 8], bf16)
            nc.sync.dma_start(padneg_b[:], kw["PADNEG"][:])
            b1t = cpool.tile([P, 64], f32)
            nc.sync.dma_start(b1t[:], kw["B1T"][:])
            as2t = cpool.tile([P, 64], f32)
            nc.sync.dma_start(as2t[:], kw["AS2T"][:])
            ad2t = cpool.tile([P, 64], f32)
            nc.sync.dma_start(ad2t[:], kw["AD2T"][:])
        else:
            gidx = cpool.tile([P, G], f32)
            nc.sync.dma_start(gidx[:], kw["GIDX"][:])
            ones = cpool.tile([P, 1], f32)
            nc.vector.memset(ones[:], 1.0)
            pp = pppool.tile([G, 65], f32)

        for w in range(W):
            rb = slice(w * P, (w + 1) * P)
            sidx = ipool.tile([P, TW], i32, tag="sidx")
            nc.sync.dma_start(sidx[:], SRC[rb, :])
            drel = ipool.tile([P, TW], f32, tag="drel")
            nc.sync.dma_start(drel[:], DREL[rb, :])
            X = npool.tile([P, TC], bf16, tag="X")
            nc.sync.dma_start(X[:], OWN[rb, :])

            # one-hot selection matrices for all tiles of this window
            S = spool.tile([P, TW * P], bf16)
            S3 = S[:].rearrange("p (t c) -> p t c", c=P)
            drel_b = bass.AP(drel[:].tensor, drel[:].offset,
                             [drel[:].ap[0], [1, TW], [0, P]])
            fidx_b = bass.AP(fidx[:].tensor, fidx[:].offset,
                             [fidx[:].ap[0], [0, TW], [1, P]])
            nc.vector.tensor_tensor(out=S3, in0=drel_b, in1=fidx_b,
                                    op=AluOp.is_equal)

            g = gpool.tile([P, TW * TC], bf16, tag="g")
            gv = g[:].rearrange("p (t c) -> p t c", c=TC)
            a = gpool.tile([P, TW * NH], f32, tag="a")
            av = a[:].rearrange("p (t c) -> p t c", c=NH)
            # adst of own (dst) nodes for this window
            ad_own = (X[:, 72:80] if layer == 1 else X[:, 65:66])
            tww = cfg["TWW"][w]
            for t in range(tww):
                nc.gpsimd.indirect_dma_start(
                    out=gv[:, t, :], out_offset=None, in_=TAB[:],
                    in_offset=bass.IndirectOffsetOnAxis(
                        ap=sidx[:, t:t + 1], axis=0))
                # adst[dst] = S_t @ adst_own, via PE transpose of S_t
                st_ps = stpool.tile([P, P], bf16, tag="st")
                nc.tensor.transpose(out=st_ps[:], in_=S[:, t * P:(t + 1) * P],
                                    identity=idn[:])
                stsb = spool.tile([P, P], bf16, tag="stsb")
                nc.scalar.copy(stsb[:], st_ps[:])
                ad_ps = adpool.tile([P, NH], f32, tag="ad")
                nc.tensor.matmul(out=ad_ps[:], lhsT=stsb[:], rhs=ad_own,
                                 start=True, stop=True)
                nc.scalar.copy(av[:, t, :], ad_ps[:])

            rhs = rpool.tile([P, TW * RC], bf16)
            # stream ae columns into the matmul rhs tile
            ae_src = AE[rb, :].rearrange("p (t c) -> p t c", c=10)
            rhs3 = rhs[:].rearrange("p (t c) -> p t c", c=RC)
            g3 = g[:].rearrange("p (t c) -> p t c", c=TC)
            a3 = a[:].rearrange("p (t c) -> p t c", c=NH)
            if layer == 1:
                nc.sync.dma_start(rhs3[:, :, 72:80], ae_src[:, :, 0:8])
                nc.sync.dma_start(rhs3[:, :, 80:81], ae_src[:, :, 9:10])
                ae_slice = rhs3[:, :, 72:80]
            else:
                nc.sync.dma_start(rhs3[:, :, 65:66], ae_src[:, :, 8:9])
                nc.sync.dma_start(rhs3[:, :, 66:67], ae_src[:, :, 9:10])
                ae_slice = rhs3[:, :, 65:66]

            # logits: u = asrc[src] + adst[dst] + ae
            nc.vector.tensor_add(a3, a3, g3[:, :, 64:64 + NH])
            nc.vector.tensor_add(a3, a3, ae_slice)
            lm = spool.tile([P, TW * NH], f32, tag="lm")
            lm3 = lm[:].rearrange("p (t c) -> p t c", c=NH)
            nc.vector.tensor_scalar(out=lm3, in0=a3, scalar1=0.0, scalar2=0.2,
                                    op0=AluOp.min, op1=AluOp.mult)
            nc.vector.scalar_tensor_tensor(out=a3, in0=a3, scalar=0.0, in1=lm3,
                                           op0=AluOp.max, op1=AluOp.add)
            ex_dst = rhs3[:, :, 64:64 + NH]
            nc.scalar.activation(ex_dst, a3, AF.Exp)

            # msg = ex (broadcast over channels) * x_src
            rr = rhs[:]
            gg = g[:]
            if layer == 1:
                out4 = bass.AP(rr.tensor, rr.offset,
                               [rr.ap[0], [RC, TW], [8, 8], [1, 8]])
                in04 = bass.AP(gg.tensor, gg.offset,
                               [gg.ap[0], [TC, TW], [8, 8], [1, 8]])
                ex4 = bass.AP(rr.tensor, rr.offset + 64,
                              [rr.ap[0], [RC, TW], [1, 8], [0, 8]])
            else:
                out4 = bass.AP(rr.tensor, rr.offset,
                               [rr.ap[0], [RC, TW], [1, 64]])
                in04 = bass.AP(gg.tensor, gg.offset,
                               [gg.ap[0], [TC, TW], [1, 64]])
                ex4 = bass.AP(rr.tensor, rr.offset + 64,
                              [rr.ap[0], [RC, TW], [0, 64]])
            nc.vector.tensor_mul(out4, in04, ex4)

            ps = pspool.tile([P, RC], f32)
            for t in range(tww):
                nc.tensor.matmul(out=ps[:], lhsT=S[:, t * P:(t + 1) * P],
                                 rhs=rhs[:, t * RC:(t + 1) * RC],
                                 start=(t == 0), stop=(t == tww - 1))

            # ---- node phase ----
            deg = npool.tile([P, 1], f32, tag="deg")
            nc.vector.tensor_scalar_max(deg[:], ps[:, RC - 1:RC], 1.0)
            rdeg = npool.tile([P, 1], f32, tag="rdeg")
            nc.vector.reciprocal(rdeg[:], deg[:])
            la = npool.tile([P, NH], f32, tag="la")
            nc.vector.tensor_scalar_mul(la[:], ps[:, RC - 1 - NH:RC - 1],
                                        rdeg[:])
            z = npool.tile([P, NH], f32, tag="z")
            nc.vector.tensor_add(z[:], X[:, 64:64 + NH],
                                 X[:, 64 + NH:64 + 2 * NH])
            nc.vector.tensor_add(z[:], z[:], la[:])
            zm = npool.tile([P, NH], f32, tag="zm")
            nc.vector.tensor_scalar(out=zm[:], in0=z[:], scalar1=0.0,
                                    scalar2=0.2, op0=AluOp.min,
                                    op1=AluOp.mult)
            nc.vector.scalar_tensor_tensor(out=z[:], in0=z[:], scalar=0.0,
                                           in1=zm[:], op0=AluOp.max,
                                           op1=AluOp.add)
            exl = npool.tile([P, NH], f32, tag="exl")
            nc.scalar.activation(exl[:], z[:], AF.Exp)
            den = npool.tile([P, NH], f32, tag="den")
            nc.vector.tensor_add(den[:], ps[:, 64:64 + NH], exl[:])
            nc.vector.tensor_scalar_add(den[:], den[:], 1.0e-16)
            rd = npool.tile([P, NH], f32, tag="rd")
            nc.vector.reciprocal(rd[:], den[:])
            num = npool.tile([P, 64], f32, tag="num")
            if layer == 1:
                exl_b = bass.AP(exl[:].tensor, exl[:].offset,
                                [exl[:].ap[0], [1, 8], [0, 8]])
                rd_b = bass.AP(rd[:].tensor, rd[:].offset,
                               [rd[:].ap[0], [1, 8], [0, 8]])
                num3 = num[:].rearrange("p (h c) -> p h c", c=8)
                nc.vector.tensor_mul(num3, X[:].rearrange(
                    "p (h c) -> p h c", c=8)[:, 0:8, :], exl_b)
                nc.vector.tensor_add(num[:], num[:], ps[:, 0:64])
                o = npool.tile([P, 64], f32, tag="o")
                nc.vector.tensor_mul(o[:].rearrange("p (h c) -> p h c", c=8),
                                     num3, rd_b)
                nc.vector.tensor_add(o[:], o[:], b1t[:])
                # ELU -> h, then asrc2/adst2
                relu_o = npool.tile([P, 64], f32, tag="relu_o")
                nc.scalar.activation(relu_o[:], o[:], AF.Relu)
                ho = npool.tile([P, 66], bf16, tag="ho")
                nc.scalar.activation(ho[:, 0:64], o[:], AF.Exp)
                nc.vector.tensor_scalar_add(ho[:, 0:64], ho[:, 0:64], -1.0)
                nc.vector.tensor_scalar_min(ho[:, 0:64], ho[:, 0:64], 0.0)
                nc.vector.tensor_add(ho[:, 0:64], ho[:, 0:64], relu_o[:])
                tmp = npool.tile([P, 64], f32, tag="tmp")
                asd = npool.tile([P, 2], f32, tag="asd")
                nc.vector.tensor_mul(tmp[:], ho[:, 0:64], as2t[:])
                nc.vector.reduce_sum(asd[:, 0:1], tmp[:],
                                     axis=mybir.AxisListType.X)
                nc.vector.tensor_mul(tmp[:], ho[:, 0:64], ad2t[:])
                nc.vector.reduce_sum(asd[:, 1:2], tmp[:],
                                     axis=mybir.AxisListType.X)
                nc.vector.tensor_copy(ho[:, 64:66], asd[:])
                if w == W - 1:
                    nc.vector.tensor_add(ho[:, 64:65], ho[:, 64:65],
                                         padneg_b[:, 0:1])
                nc.sync.dma_start(kw["HOUT"][rb, :], ho[:])
            else:
                nc.vector.tensor_scalar_mul(num[:], X[:, 0:64], exl[:])
                nc.vector.tensor_add(num[:], num[:], ps[:, 0:64])
                agg = npool.tile([P, 65], f32, tag="agg")
                nc.vector.tensor_scalar_mul(agg[:, 0:64], num[:], rd[:])
                nc.vector.tensor_copy(agg[:, 64:65], ones[:])
                brel = ipool.tile([P, 1], f32, tag="brel")
                nc.sync.dma_start(brel[:], kw["BREL"][rb, :])
                sg = spool.tile([P, G], f32, tag="sg")
                brel_b = bass.AP(brel[:].tensor, brel[:].offset,
                                 [brel[:].ap[0], [0, G]])
                nc.vector.tensor_tensor(out=sg[:], in0=brel_b, in1=gidx[:],
                                        op=AluOp.is_equal)
                nc.tensor.matmul(out=pp[:], lhsT=sg[:], rhs=agg[:],
                                 start=(w == 0), stop=(w == W - 1))

        if layer == 2:
            ppsb = npool.tile([G, 65], f32, tag="ppsb")
            nc.scalar.copy(ppsb[:], pp[:])
            nc.sync.dma_start(kw["QIN"][:], ppsb[:])


# ----------------------------------------------------------------------------
# PJRT exec wrapper: stage once, run warm, time honestly
# ----------------------------------------------------------------------------

def _make_exec(nc, n_cores):
    import jax
    from jax.sharding import Mesh, PartitionSpec, NamedSharding
    try:
        from jax import shard_map
    except ImportError:
        from jax.experimental.shard_map import shard_map
    import concourse.mybir as mybir
    from concourse import bass2jax
    from concourse.bass2jax import _bass_exec_p, install_neuronx_cc_hook

    install_neuronx_cc_hook()
    partition_name = (nc.partition_id_tensor.name
                      if nc.partition_id_tensor else None)
    in_names, out_names, out_avals, zero_outs = [], [], [], []
    for alloc in nc.m.functions[0].allocations:
        if not isinstance(alloc, mybir.MemoryLocationSet):
            continue
        name = alloc.memorylocations[0].name
        if alloc.kind == "ExternalInput":
            if name != partition_name:
                in_names.append(name)
        elif alloc.kind == "ExternalOutput":
            shape = tuple(alloc.tensor_shape)
            dtype = mybir.dt.np(alloc.dtype)
            out_names.append(name)
            out_avals.append(jax.core.ShapedArray(shape, dtype))
            zero_outs.append(np.zeros(shape, dtype))
    n_params = len(in_names)
    n_outs = len(out_avals)
    all_names = list(in_names) + list(out_names)
    if partition_name is not None:
        all_names.append(partition_name)
    donate = tuple(range(n_params, n_params + n_outs))

    def _body(*args):
        operands = list(args)
        if partition_name is not None:
            operands.append(bass2jax.partition_id_tensor())
        outs = _bass_exec_p.bind(
            *operands,
            out_avals=tuple(out_avals),
            in_names=tuple(all_names),
            out_names=tuple(out_names),
            lowering_input_output_aliases=(),
            sim_require_finite=True,
            sim_require_nnan=True,
            nc=nc,
        )
        return tuple(outs)

    devices = jax.devices()[:n_cores]
    assert len(devices) == n_cores, \
        f"need {n_cores} devices, have {len(jax.devices())}"
    mesh = Mesh(np.asarray(devices), ("core",))
    in_specs = (PartitionSpec("core"),) * (n_params + n_outs)
    out_specs = (PartitionSpec("core"),) * n_outs
    try:
        smapped = shard_map(_body, mesh=mesh, in_specs=in_specs,
                            out_specs=out_specs, check_vma=False)
    except TypeError:
        smapped = shard_map(_body, mesh=mesh, in_specs=in_specs,
                            out_specs=out_specs, check_rep=False)
    sharded = jax.jit(smapped, donate_argnums=donate, keep_unused=True)
    sh = NamedSharding(mesh, PartitionSpec("core"))
    return dict(fn=sharded, in_names=in_names, out_names=out_names,
                out_avals=out_avals, zero_outs=zero_outs, sh=sh, jax=jax)


def _run_fused(nc, in_maps, n_cores, timing=None, n_timed=3):
    ex = _make_exec(nc, n_cores)
    jax = ex["jax"]
    sh = ex["sh"]

    dev_in = [jax.device_put(
        np.concatenate([np.asarray(in_maps[c][n]) for c in range(n_cores)],
                       axis=0), sh)
        for n in ex["in_names"]]
    for a in dev_in:
        a.block_until_ready()

    def zeros():
        zs = [jax.device_put(
            np.zeros((n_cores * z.shape[0], *z.shape[1:]), z.dtype), sh)
            for z in ex["zero_outs"]]
        for a in zs:
            a.block_until_ready()
        return zs

    # cold call: NEFF compile + load happen here
    outs = ex["fn"](*dev_in, *zeros())
    for o in outs:
        o.block_until_ready()

    walls = []
    for _ in range(n_timed):
        z = zeros()
        t0 = time.time()
        outs = ex["fn"](*dev_in, *z)
        for o in outs:
            o.block_until_ready()
        walls.append(time.time() - t0)

    # Pipelined batches of K back-to-back executions. Executions serialize
    # on-device, so wall(K) = fixed_dispatch + K * t_exec; the slope of
    # wall(K) vs K is the per-execution hardware time with host dispatch
    # overhead excluded (the closest available proxy for NTFF exec time).
    def batch(K):
        zsets = [zeros() for _ in range(K)]
        t0 = time.time()
        all_outs = [ex["fn"](*dev_in, *z) for z in zsets]
        for o in all_outs[-1]:
            o.block_until_ready()
        return time.time() - t0, all_outs[-1]

    ks = [2, 12, 22]
    slopes = []
    batches = []
    for _ in range(3):
        sweep = []
        for K in ks:
            wall, outs = batch(K)
            sweep.append((K, wall))
        km = sum(k for k, _ in sweep) / len(sweep)
        wm = sum(w for _, w in sweep) / len(sweep)
        slopes.append(sum((k - km) * (w - wm) for k, w in sweep)
                      / sum((k - km) ** 2 for k, _ in sweep))
        batches.extend(sweep)
    amortized = min(w / k for k, w in batches)
    valid = [s for s in slopes if s > 0]
    t_exec = min(valid + [amortized]) if valid else amortized

    np_outs = [np.asarray(o) for o in outs]
    results = []
    for c in range(n_cores):
        m = {}
        for i, name in enumerate(ex["out_names"]):
            shape = ex["out_avals"][i].shape
            m[name] = np_outs[i].reshape(n_cores, *shape)[c]
        results.append(m)
    if timing is not None:
        timing["warm_walls_s"] = walls
        timing["pipe_batches"] = batches
        timing["amortized_s"] = amortized
        timing["slope_s"] = slopes
        timing["exec_ns"] = [int(t_exec * 1e9)]
    return results


# ----------------------------------------------------------------------------
# Top-level kernel
# ----------------------------------------------------------------------------

def kernel(x, edge_index, batch, edge_attr, W1, We1, att_src1, att_dst1,
           att_edge1, b1, W2, We2, att_src2, att_dst2, att_edge2, b2,
           ncores=NCORES, _timing=None):
    x = np.asarray(x, np.float32)
    n = x.shape[0]
    batch = np.asarray(batch, np.int64)
    G = G_FULL

    prep = _prep_graph(edge_index, edge_attr, n, ncores)
    W, TW, cn = prep["W"], prep["TW"], prep["cn"]
    WP = W * P

    W1E, AEE, As2, Ad2, W2f = _combine_weights(
        W1, att_src1, att_dst1, att_edge1, We1,
        W2, att_src2, att_dst2, att_edge2, We2)
    b1f = np.asarray(b1, np.float32)
    b2f = np.asarray(b2, np.float32)

    FIDX = np.broadcast_to(np.arange(P, dtype=np.float32), (P, P)).copy()
    GIDX = np.broadcast_to(np.arange(G, dtype=np.float32), (P, G)).copy()
    B1T = np.broadcast_to(b1f, (P, 64)).copy()
    AS2T = np.broadcast_to(As2, (P, 64)).copy()
    AD2T = np.broadcast_to(Ad2, (P, 64)).copy()
    import ml_dtypes
    IDN = np.eye(P, dtype=np.float32)
    B2T = np.broadcast_to(b2f, (G, 64)).copy()
    PADROW = cn - WP + P
    PADNEG = np.zeros((P, 8), np.float32)
    PADNEG[PADROW:, :] = NEG
    PADNEG = PADNEG.astype(ml_dtypes.bfloat16)
    IDNH = np.eye(P, dtype=ml_dtypes.bfloat16)

    # batch_rel per core (pad -1 past real nodes)
    BRELs = []
    for c in range(ncores):
        br = np.full((WP, 1), -1.0, np.float32)
        real = batch[c * cn:(c + 1) * cn].astype(np.float32)
        br[: real.shape[0], 0] = real
        BRELs.append(br)

    # x^T slices padded to W*P cols
    xT = x.T  # [128, n]
    XTs = []
    for c in range(ncores):
        xt = np.zeros((P, WP), np.float32)
        lo = c * cn
        hi = min(lo + WP, n)
        xt[:, : hi - lo] = xT[:, lo:hi]
        XTs.append(xt)

    cfg = dict(W=W, TW=TW, G=G, cn=cn, TWW=prep["TWW"])

    t0 = time.time()
    nc = build_fused(cfg)
    t1 = time.time()

    in_maps = [dict(XT=XTs[c], W1E=W1E, EAPD=prep["per_core"][c]["EAP"],
                    AEE=AEE, SRC=prep["per_core"][c]["SRC"],
                    DST=prep["per_core"][c]["DST"],
                    DREL=prep["per_core"][c]["DREL"],
                    FIDX=FIDX, B1T=B1T, AS2T=AS2T, AD2T=AD2T,
                    BREL=BRELs[c], GIDX=GIDX, W2T=W2f, IDN=IDN, B2T=B2T,
                    PADNEG=PADNEG, IDNH=IDNH)
               for c in range(ncores)]

    results = _run_fused(nc, in_maps, ncores, timing=_timing)
    if _timing is not None:
        _timing["build_s"] = t1 - t0

    return results[0]["POOL"].astype(np.float32)
